# revision 1
# baseline (speedup 1.0000x reference)
"""Depthwise morphological (max-plus) dilation, 3x3, stride 1, zero-pad 1.

out[b,c,i,j] = max_{p,q} ( x_pad[b,c,i+p,j+q] + se[c,p,q] )

Sharding: pure data parallel over batch (16 batches -> 8 cores x 2).
On-core layout: partition dim = 2 batches x 64 channels = 128 planes;
each partition processes its own plane in row-blocks of R=32 output
rows. The host supplies x zero-padded to [P, H+2, W+2] and converted to
fp16, so all 9 taps are free-dim-shifted 3D views of one SBUF tile and
the device does no zero-fill.

Engine schedule (per output element: 9 scalar-adds + 8 tensor-maxes):
  DVE  tensor_scalar add runs in 4x perf mode   (~0.26 ns/elem/part)
  DVE  tensor_tensor max runs in 2x perf mode   (~0.52 ns/elem/part)
  ACT  activation-bias add, 1 elem/cycle @1.2GHz (~0.83 ns/elem/part)
GpSimd/Pool cannot run elementwise ops on this backend (ISA check
rejects TensorTensor on Pool), and the fused scalar_tensor_tensor loses
all DVE perf modes, so the optimum is unfused ops split DVE/ACT:
3 adds + 8 maxes on DVE (~4.95 ns/elem) and 6 adds on ACT (~5.0),
i.e. both engines balanced at the ~325 us/core compute floor
(memory roofline is ~95 us; 17 exact elementwise ops/elem bind first).
fp16 compute gives ~6e-4 max rel err vs the fp32 reference.
"""

import numpy as np

B, C, H, W = 16, 64, 256, 256
K = 3
NCORES = 8
BPC = B // NCORES          # batches per core
P = BPC * C                # 128 partitions
HP, WP = H + 2, W + 2      # host-padded plane

COMPUTE = "f16"            # "f16" (fast, ~6e-4 rel err) or "f32" (exact)
R = 32                     # output rows per block
DVE_TAPS = (0, 5, 8)       # tap t = di*3+dj; DVE does these adds (t0=root)
ACT_TAPS = (1, 2, 3, 4, 6, 7)  # adds on ScalarE; maxes on DVE

_prog_cache = {}


def _build(compute=COMPUTE, h=H, r=R, reps=1,
           dve_taps=DVE_TAPS, act_taps=ACT_TAPS,
           xbufs=2, abufs=2, atmp_bufs=7, dtmp_bufs=1, dtmp_tag="dtmp",
           split=None, flat=False):
    """Build the Bass program for one core: x [P,h+2,W+2] -> o [P,h,W]."""
    import concourse.bacc as bacc
    import concourse.mybir as mybir
    from concourse.tile import TileContext

    add, mx = mybir.AluOpType.add, mybir.AluOpType.max
    dt = mybir.dt.float16 if compute == "f16" else mybir.dt.float32

    nc = bacc.Bacc()
    x_d = nc.dram_tensor("x", [P, h + 2, W + 2], dt, kind="ExternalInput")
    se_d = nc.dram_tensor("se", [P, K * K], mybir.dt.float32, kind="ExternalInput")
    o_d = nc.dram_tensor("o", [P, h, W], dt, kind="ExternalOutput")

    assert len(dve_taps) + len(act_taps) == K * K

    with TileContext(nc) as tc:
        with (
            tc.tile_pool(name="cpool", bufs=1) as cpool,
            tc.tile_pool(name="xpool", bufs=xbufs) as xpool,
            tc.tile_pool(name="apool", bufs=abufs) as apool,
            tc.tile_pool(name="tpool", bufs=2) as tpool,
        ):
            se_sb = cpool.tile([P, K * K], mybir.dt.float32)
            nc.sync.dma_start(out=se_sb[:], in_=se_d[:, :])

            for r0 in [v for _ in range(reps) for v in range(0, h, r)]:
                xt = xpool.tile([P, r + 2, W + 2], dt, tag="xt")
                nc.sync.dma_start(out=xt[:], in_=x_d[:, r0 : r0 + r + 2, :])

                if flat:
                    # 1-D views over the padded tile: every tap shift is a
                    # constant flat offset (di*(W+2)+dj). Ops cover
                    # r*(W+2)-2 elems incl. 2 junk columns/row never stored
                    # (-2 so the largest-offset tap stays inside the tile).
                    L = r * (W + 2) - 2
                    acc = apool.tile([P, r, W + 2], dt, tag="acc")
                    xf = xt[:].rearrange("p h w -> p (h w)")

                    def vw(tile):
                        return tile[:].rearrange("p h w -> p (h w)")[:, 0:L]

                    def src(t):
                        di, dj = divmod(t, K)
                        off = di * (W + 2) + dj
                        return xf[:, off : off + L]
                else:
                    acc = apool.tile([P, r, W], dt, tag="acc")

                    def vw(tile):
                        return tile[:]

                    def src(t):
                        di, dj = divmod(t, K)
                        return xt[:, di : di + r, dj : dj + W]

                def sca(t):
                    return se_sb[:, t : t + 1]

                # All ACT adds issued at block start so ACT paces itself a
                # full block's worth of addends (6 bufs); the DVE chain then
                # runs its own links first and consumes the ACT tmps last.
                # split=(tap, s): DVE computes the last s rows of that ACT
                # tap's addend — fractional load-balance between the engines.
                tshape = [P, r, W + 2] if flat else [P, r, W]
                act_tmps = {}
                for t in act_taps:
                    tmp = tpool.tile(tshape, dt, tag="atmp", bufs=atmp_bufs)
                    if split and t == split[0]:
                        nc.scalar.add(tmp[:, : r - split[1], :],
                                      src(t)[:, : r - split[1], :], sca(t))
                    else:
                        nc.scalar.add(vw(tmp), src(t), sca(t))
                    act_tmps[t] = tmp
                t0 = dve_taps[0]
                nc.vector.tensor_scalar(vw(acc), src(t0), sca(t0), None, add)
                if split:
                    tsp, s = split
                    nc.vector.tensor_scalar(
                        act_tmps[tsp][:, r - s :, :],
                        src(tsp)[:, r - s :, :], sca(tsp), None, add)
                for t in dve_taps[1:]:
                    tb = atmp_bufs if dtmp_tag == "atmp" else dtmp_bufs
                    tmp = tpool.tile(tshape, dt, tag=dtmp_tag, bufs=tb)
                    nc.vector.tensor_scalar(vw(tmp), src(t), sca(t), None, add)
                    nc.vector.tensor_tensor(vw(acc), vw(acc), vw(tmp), mx)
                for t in act_taps:
                    nc.vector.tensor_tensor(vw(acc), vw(acc), vw(act_tmps[t]), mx)

                st = acc[:, :, 0:W] if flat else acc[:]
                nc.sync.dma_start(out=o_d[:, r0 : r0 + r, :], in_=st)
    # bacc legalization (splits >1-wait instructions into event semaphores)
    nc.finalize()
    return nc


def _get_prog(key=("default",)):
    if key not in _prog_cache:
        _prog_cache[key] = _build()
    return _prog_cache[key]


def _pad_shard(x_shard, np_dt):
    """[BPC,C,H,W] fp32 -> zero-padded [P, H+2, W+2] in np_dt."""
    xp = np.zeros((P, HP, WP), np_dt)
    xp[:, 1 : H + 1, 1 : W + 1] = x_shard.reshape(P, H, W)
    return xp


def _run(x, se, **spmd_kwargs):
    from concourse.bass_utils import run_bass_kernel_spmd

    nc = _get_prog()
    np_dt = np.float16 if COMPUTE == "f16" else np.float32
    x = np.asarray(x)
    se_p = np.tile(np.asarray(se, np.float32).reshape(C, K * K), (BPC, 1))
    in_maps = [
        {"x": _pad_shard(x[k * BPC : (k + 1) * BPC], np_dt), "se": se_p}
        for k in range(NCORES)
    ]
    res = run_bass_kernel_spmd(nc, in_maps, core_ids=list(range(NCORES)), **spmd_kwargs)
    out = np.empty((B, C, H, W), np.float32)
    for k in range(NCORES):
        out[k * BPC : (k + 1) * BPC] = (
            res.results[k]["o"].astype(np.float32).reshape(BPC, C, H, W)
        )
    return out, res


def kernel(x: np.ndarray, se: np.ndarray) -> np.ndarray:
    return _run(x, se)[0]



# revision 2
# speedup vs baseline: 1.5724x; 1.5724x over previous
"""Depthwise morphological (max-plus) dilation, 3x3, stride 1, zero-pad 1.

out[b,c,i,j] = max_{p,q} ( x_pad[b,c,i+p,j+q] + se[c,p,q] )

Sharding: pure data parallel over batch (16 batches -> 8 cores x 2).
On-core layout: partition dim = 2 batches x 64 channels = 128 planes;
each partition processes its own plane in row-blocks of R output rows.
The host supplies x zero-padded to [P, H+2, W+2] in fp16 plus a derived
per-partition SE tensor, so the device does no zero-fill.

Engine schedule — hand-written custom DVE microcode (3 uops/elem
instead of 17 stock ALU ops/elem):
  ACT   y_p = x + se[c,p,1]          (3 bias adds, rows p=0,1,2)
  DVE   h   = HDIL3(y_0)             out[k]=max(y[k]+d02, y[k-1], y[k-2]+d00)
  DVE   t   = HDIL3ACC(y_1, h)       ... max(..., h[k])
  DVE   o   = HDIL3ACC(y_2, t)
HDIL3* are custom DVE ops (registered into concourse.dve_ops at import):
the 3-tap sliding window lives in per-stage swap-flop delays (BYPASS
outputs operand a = the flop's previous value while capturing operand
b), so each op streams at 1 elem/cycle — the middle tap's constant is
folded into the ACT pre-add, the outer taps' deltas ride the two
per-partition scalar slots. DVE: 3 cyc/elem (~205us/core), ACT: 3
ops/elem (~180us/core), DMA ~95us/core — vs ~325us/core for the best
stock-op split (DVE 4x/2x perf-mode add/max chain).

fp16 compute gives ~2e-3 max abs err vs the fp32 reference (scale ~6).
"""

import numpy as np

B, C, H, W = 16, 64, 256, 256
K = 3
NCORES = 8
BPC = B // NCORES          # batches per core
P = BPC * C                # 128 partitions
HP, WP = H + 2, W + 2      # host-padded plane

COMPUTE = "f16"            # "f16" (fast, ~2e-3 abs err) or "f32" (exact)
R = 16                     # output rows per block

_prog_cache = {}

# --------------------------------------------------------------------------
# Custom DVE ops: 3-tap sliding-window max-plus via swap-flop delays.
#   HDIL3_ANT:    out[k] = max(y[k]+C0, y[k-1], y[k-2]+C1)
#   HDIL3ACC_ANT: out[k] = max(y[k]+C0, y[k-1], y[k-2]+C1, z[k])
# C0/C1 are per-partition scalars (s0/s1 APs). First two outputs of each
# instruction are stale-flop garbage; the stream layout keeps those in
# never-stored junk columns.
# --------------------------------------------------------------------------


class _HandDveOp:
    """Duck-typed stand-in for dve_ops.DveOp with hand-written uops."""

    def __init__(self, name, spec, built, subdim=False):
        self.name, self.spec, self.subdim = name, spec, subdim
        self._built = built

    def compile(self, ver):
        assert ver == "v3", f"hand op {self.name} only built for v3, got {ver}"
        return self._built


def _mk_hdil3_uop(acc):
    from concourse.dve_uop import (
        UopConfig, AluOp, AluInp, InpSel, OutSel, OutPath, Trigger,
        DelayInp, ENABLE,
    )

    u = UopConfig()
    u.enable_input(InpSel.SRC_0, 1)    # chain0 = y[k]
    u.enable_input(InpSel.CONST_0, 2)  # chain1 = C0 (tap k   / SE col 2)
    u.enable_input(InpSel.CONST_1, 3)  # chain2 = C1 (tap k-2 / SE col 0)
    if acc:
        u.enable_input(InpSel.SRC_1, 4)  # chain3 = z[k]
        u.require_inp1 = ENABLE
    u.require_inp0 = ENABLE
    u.trigger = (Trigger.SRC_TENSOR_DONE, Trigger.NONE, Trigger.NONE)
    dp = u.datapath_config

    for st in range(8):
        dp[st].pass_through_delay(*([0, 1, 2] + ([3] if acc else [])))
    dly = 4 if acc else 3   # chain carrying y[k-1]
    mch = 5 if acc else 4   # chain carrying m0

    def delay_block(blk, src1):
        # out = swap_prev; swap <- operand b  (1-element delay)
        blk.op = AluOp.BYPASS
        blk.alu_src0 = AluInp.CURR_SWAP_OUT
        blk.alu_src1 = src1
        blk.alu_out_enable = ENABLE
        blk.swap_enable = ENABLE

    delay_block(dp[0], AluInp.PREV_DELAY_0)                  # y[k-1]
    delay_block(dp[1], AluInp.PREV_ALU_OUT)                  # y[k-2]
    dp[1].enable_delay_from_src(DelayInp.PREV_ALU_OUT, dly)  # chain <- y[k-1]
    dp[2].enable_alu(AluOp.ADD, AluInp.PREV_ALU_OUT, AluInp.PREV_DELAY_2)
    dp[2].pass_through_delay(dly)
    dp[3].enable_alu(AluOp.MAX, AluInp.PREV_ALU_OUT,
                     AluInp(AluInp.PREV_DELAY_0 + dly))      # max(., y[k-1])
    dp[4].enable_alu(AluOp.ADD, AluInp.PREV_DELAY_0, AluInp.PREV_DELAY_1)
    dp[4].enable_delay_from_src(DelayInp.PREV_ALU_OUT, mch)  # chain <- m0
    dp[5].enable_alu(AluOp.MAX, AluInp.PREV_ALU_OUT,
                     AluInp(AluInp.PREV_DELAY_0 + mch))      # max(y[k]+C0, m0)
    if acc:
        dp[6].enable_alu(AluOp.MAX, AluInp.PREV_ALU_OUT, AluInp.PREV_DELAY_3)
    else:
        dp[6].pass_through_alu()
    dp[7].pass_through_alu()
    u.enable_output(OutSel.ALU_OUT, OutPath.WR0_LO)
    return u


def _hdil3_core(y, c0, c1):
    out = np.empty_like(y)
    out[..., 2:] = np.maximum(
        np.maximum(y[..., 2:] + c0, y[..., 1:-1]), y[..., :-2] + c1)
    out[..., :2] = 0.0  # HW: stale flop garbage
    return out


def _get_hdil_ops():
    import concourse.dve_ops as dve_ops_mod
    from concourse.dve_spec import Spec, Src0, Src1
    from concourse.dve_uop import DveOpSpec

    def ref1(in0, in1, s0, s1, imm2):
        return _hdil3_core(in0, np.asarray(s0)[..., None],
                           np.asarray(s1)[..., None])

    def ref2(in0, in1, s0, s1, imm2):
        r = ref1(in0, None, s0, s1, imm2)
        r[..., 2:] = np.maximum(r[..., 2:], in1[..., 2:])
        return r

    out = []
    for name, acc, ref in (("HDIL3_ANT", False, ref1),
                           ("HDIL3ACC_ANT", True, ref2)):
        if name in dve_ops_mod._SUB_OPCODE_FOR_NAME:
            out.append(next(o for o in dve_ops_mod.OPS if o.name == name))
            continue
        row = max(dve_ops_mod._SUB_OPCODE_FOR_NAME.values()) + 1
        assert row < 0x20
        built = DveOpSpec(name=name, uops=[_mk_hdil3_uop(acc)],
                          opcode=row, rd1_en=acc)
        built.validate("v3")
        op = _HandDveOp(name, Spec(body=Src0 + Src1 if acc else Src0,
                                   reference=ref), built)
        dve_ops_mod.OPS.append(op)
        dve_ops_mod._SUB_OPCODE_FOR_NAME[name] = row
        dve_ops_mod.CUSTOM_DVE_SPECS[name] = op.spec
        out.append(op)
    return out


# --------------------------------------------------------------------------
# Bass program
# --------------------------------------------------------------------------


def _build(compute=COMPUTE, h=H, r=R, reps=1):
    """Build the Bass program for one core: x [P,h+2,W+2] -> o [P,h,W]."""
    import concourse.bacc as bacc
    import concourse.mybir as mybir
    from concourse.tile import TileContext

    hdil3, hdil3acc = _get_hdil_ops()
    dt = mybir.dt.float16 if compute == "f16" else mybir.dt.float32
    f32 = mybir.dt.float32

    nc = bacc.Bacc()
    x_d = nc.dram_tensor("x", [P, h + 2, W + 2], dt, kind="ExternalInput")
    # se2[:, 3p+0] = se[c,p,1]; 3p+1 = se[c,p,2]-se[c,p,1]; 3p+2 = se[c,p,0]-se[c,p,1]
    se_d = nc.dram_tensor("se", [P, K * K], f32, kind="ExternalInput")
    o_d = nc.dram_tensor("o", [P, h, W], dt, kind="ExternalOutput")

    with TileContext(nc) as tc:
        with (
            tc.tile_pool(name="cpool", bufs=1) as cpool,
            tc.tile_pool(name="xpool", bufs=2) as xpool,
            tc.tile_pool(name="ypool", bufs=2) as ypool,
            tc.tile_pool(name="apool", bufs=2) as apool,
        ):
            se_sb = cpool.tile([P, K * K], f32)
            nc.sync.dma_start(out=se_sb[:], in_=se_d[:, :])

            L = r * (W + 2)
            for r0 in [v for _ in range(reps) for v in range(0, h, r)]:
                xt = xpool.tile([P, r + 2, W + 2], dt, tag="xt")
                nc.sync.dma_start(out=xt[:], in_=x_d[:, r0 : r0 + r + 2, :])

                ys = []
                for p in range(3):
                    y = ypool.tile([P, r, W + 2], dt, tag=f"y{p}")
                    nc.scalar.add(y[:], xt[:, p : p + r, :],
                                  se_sb[:, 3 * p : 3 * p + 1])
                    ys.append(y)

                def flat(tile):
                    return tile[:].rearrange("p h w -> p (h w)")

                acc = None
                for p in range(3):
                    nxt = apool.tile([P, r, W + 2], dt, tag=f"a{p}", bufs=2)
                    kw = dict(
                        out=flat(nxt), in0=flat(ys[p]),
                        s0=se_sb[:, 3 * p + 1 : 3 * p + 2],
                        s1=se_sb[:, 3 * p + 2 : 3 * p + 3])
                    if acc is None:
                        nc.vector._custom_dve(hdil3, **kw)
                    else:
                        nc.vector._custom_dve(hdil3acc, in1=flat(acc), **kw)
                    acc = nxt

                nc.sync.dma_start(out=o_d[:, r0 : r0 + r, :],
                                  in_=acc[:, :, 2 : W + 2])
    nc.finalize()
    return nc


def _get_prog(key=("default",)):
    if key not in _prog_cache:
        _prog_cache[key] = _build()
    return _prog_cache[key]


def _pad_shard(x_shard, np_dt):
    """[BPC,C,H,W] fp32 -> zero-padded [P, H+2, W+2] in np_dt."""
    xp = np.zeros((P, HP, WP), np_dt)
    xp[:, 1 : H + 1, 1 : W + 1] = x_shard.reshape(P, H, W)
    return xp


def _prep_se(se):
    """se [C,3,3] fp32 -> derived per-partition consts [P, 9] fp32."""
    se = np.asarray(se, np.float32)
    d = np.empty((C, K * K), np.float32)
    for p in range(K):
        d[:, 3 * p + 0] = se[:, p, 1]
        d[:, 3 * p + 1] = se[:, p, 2] - se[:, p, 1]
        d[:, 3 * p + 2] = se[:, p, 0] - se[:, p, 1]
    return np.tile(d, (BPC, 1))


def _run(x, se, **spmd_kwargs):
    from concourse.bass_utils import run_bass_kernel_spmd

    nc = _get_prog()
    np_dt = np.float16 if COMPUTE == "f16" else np.float32
    x = np.asarray(x)
    se_p = _prep_se(se)
    in_maps = [
        {"x": _pad_shard(x[k * BPC : (k + 1) * BPC], np_dt), "se": se_p}
        for k in range(NCORES)
    ]
    res = run_bass_kernel_spmd(nc, in_maps, core_ids=list(range(NCORES)), **spmd_kwargs)
    out = np.empty((B, C, H, W), np.float32)
    for k in range(NCORES):
        out[k * BPC : (k + 1) * BPC] = (
            res.results[k]["o"].astype(np.float32).reshape(BPC, C, H, W)
        )
    return out, res


def kernel(x: np.ndarray, se: np.ndarray) -> np.ndarray:
    return _run(x, se)[0]


# revision 7
# speedup vs baseline: 1.5730x; 1.0004x over previous
"""Depthwise morphological (max-plus) dilation, 3x3, stride 1, zero-pad 1.

out[b,c,i,j] = max_{p,q} ( x_pad[b,c,i+p,j+q] + se[c,p,q] )

Sharding: pure data parallel over batch (16 batches -> 8 cores x 2).
On-core layout: partition dim = 2 batches x 64 channels = 128 planes;
each partition processes its own plane in row-blocks of R output rows.
The host supplies x zero-padded to [P, H+2, W+2] in fp16 plus a derived
per-partition SE tensor, so the device does no zero-fill.

Engine schedule — hand-written custom DVE microcode (3 uops/elem
instead of 17 stock ALU ops/elem):
  ACT   y_p = x + se[c,p,1]          (3 bias adds, rows p=0,1,2)
  DVE   h   = HDIL3(y_0)             out[k]=max(y[k]+d02, y[k-1], y[k-2]+d00)
  DVE   t   = HDIL3ACC(y_1, h)       ... max(..., h[k])
  DVE   o   = HDIL3ACC(y_2, t)
HDIL3* are custom DVE ops (registered into concourse.dve_ops at import):
the 3-tap sliding window lives in per-stage swap-flop delays (BYPASS
outputs operand a = the flop's previous value while capturing operand
b), so each op streams at 1 elem/cycle — the middle tap's constant is
folded into the ACT pre-add, the outer taps' deltas ride the two
per-partition scalar slots. DVE: 3 cyc/elem (~205us/core), ACT: 3
ops/elem (~180us/core), DMA ~95us/core — vs ~325us/core for the best
stock-op split (DVE 4x/2x perf-mode add/max chain).

fp16 compute gives ~2e-3 max abs err vs the fp32 reference (scale ~6).
"""

import numpy as np

B, C, H, W = 16, 64, 256, 256
K = 3
NCORES = 8
BPC = B // NCORES          # batches per core
P = BPC * C                # 128 partitions
HP, WP = H + 2, W + 2      # host-padded plane

COMPUTE = "f16"            # "f16" (fast, ~2e-3 abs err) or "f32" (exact)
R = 32                     # output rows per block
HBUFS = 1                  # bufs for the h/t intermediate tiles

_prog_cache = {}

# --------------------------------------------------------------------------
# Custom DVE ops: 3-tap sliding-window max-plus via swap-flop delays.
#   HDIL3_ANT:    out[k] = max(y[k]+C0, y[k-1], y[k-2]+C1)
#   HDIL3ACC_ANT: out[k] = max(y[k]+C0, y[k-1], y[k-2]+C1, z[k])
# C0/C1 are per-partition scalars (s0/s1 APs). First two outputs of each
# instruction are stale-flop garbage; the stream layout keeps those in
# never-stored junk columns.
# --------------------------------------------------------------------------


class _HandDveOp:
    """Duck-typed stand-in for dve_ops.DveOp with hand-written uops."""

    def __init__(self, name, spec, built, subdim=False):
        self.name, self.spec, self.subdim = name, spec, subdim
        self._built = built

    def compile(self, ver):
        assert ver == "v3", f"hand op {self.name} only built for v3, got {ver}"
        return self._built


def _mk_hdil3_uop(acc):
    from concourse.dve_uop import (
        UopConfig, AluOp, AluInp, InpSel, OutSel, OutPath, Trigger,
        DelayInp, ENABLE,
    )

    u = UopConfig()
    u.enable_input(InpSel.SRC_0, 1)    # chain0 = y[k]
    u.enable_input(InpSel.CONST_0, 2)  # chain1 = C0 (tap k   / SE col 2)
    u.enable_input(InpSel.CONST_1, 3)  # chain2 = C1 (tap k-2 / SE col 0)
    if acc:
        u.enable_input(InpSel.SRC_1, 4)  # chain3 = z[k]
        u.require_inp1 = ENABLE
    u.require_inp0 = ENABLE
    u.trigger = (Trigger.SRC_TENSOR_DONE, Trigger.NONE, Trigger.NONE)
    dp = u.datapath_config

    for st in range(8):
        dp[st].pass_through_delay(*([0, 1, 2] + ([3] if acc else [])))
    dly = 4 if acc else 3   # chain carrying y[k-1]
    mch = 5 if acc else 4   # chain carrying m0

    def delay_block(blk, src1):
        # out = swap_prev; swap <- operand b  (1-element delay)
        blk.op = AluOp.BYPASS
        blk.alu_src0 = AluInp.CURR_SWAP_OUT
        blk.alu_src1 = src1
        blk.alu_out_enable = ENABLE
        blk.swap_enable = ENABLE

    delay_block(dp[0], AluInp.PREV_DELAY_0)                  # y[k-1]
    delay_block(dp[1], AluInp.PREV_ALU_OUT)                  # y[k-2]
    dp[1].enable_delay_from_src(DelayInp.PREV_ALU_OUT, dly)  # chain <- y[k-1]
    dp[2].enable_alu(AluOp.ADD, AluInp.PREV_ALU_OUT, AluInp.PREV_DELAY_2)
    dp[2].pass_through_delay(dly)
    dp[3].enable_alu(AluOp.MAX, AluInp.PREV_ALU_OUT,
                     AluInp(AluInp.PREV_DELAY_0 + dly))      # max(., y[k-1])
    dp[4].enable_alu(AluOp.ADD, AluInp.PREV_DELAY_0, AluInp.PREV_DELAY_1)
    dp[4].enable_delay_from_src(DelayInp.PREV_ALU_OUT, mch)  # chain <- m0
    dp[5].enable_alu(AluOp.MAX, AluInp.PREV_ALU_OUT,
                     AluInp(AluInp.PREV_DELAY_0 + mch))      # max(y[k]+C0, m0)
    if acc:
        dp[6].enable_alu(AluOp.MAX, AluInp.PREV_ALU_OUT, AluInp.PREV_DELAY_3)
    else:
        dp[6].pass_through_alu()
    dp[7].pass_through_alu()
    u.enable_output(OutSel.ALU_OUT, OutPath.WR0_LO)
    return u


def _hdil3_core(y, c0, c1):
    out = np.empty_like(y)
    out[..., 2:] = np.maximum(
        np.maximum(y[..., 2:] + c0, y[..., 1:-1]), y[..., :-2] + c1)
    out[..., :2] = 0.0  # HW: stale flop garbage
    return out


def _get_hdil_ops():
    import concourse.dve_ops as dve_ops_mod
    from concourse.dve_spec import Spec, Src0, Src1
    from concourse.dve_uop import DveOpSpec

    def ref1(in0, in1, s0, s1, imm2):
        return _hdil3_core(in0, np.asarray(s0)[..., None],
                           np.asarray(s1)[..., None])

    def ref2(in0, in1, s0, s1, imm2):
        r = ref1(in0, None, s0, s1, imm2)
        r[..., 2:] = np.maximum(r[..., 2:], in1[..., 2:])
        return r

    out = []
    for name, acc, ref in (("HDIL3_ANT", False, ref1),
                           ("HDIL3ACC_ANT", True, ref2)):
        if name in dve_ops_mod._SUB_OPCODE_FOR_NAME:
            out.append(next(o for o in dve_ops_mod.OPS if o.name == name))
            continue
        row = max(dve_ops_mod._SUB_OPCODE_FOR_NAME.values()) + 1
        assert row < 0x20
        built = DveOpSpec(name=name, uops=[_mk_hdil3_uop(acc)],
                          opcode=row, rd1_en=acc)
        built.validate("v3")
        op = _HandDveOp(name, Spec(body=Src0 + Src1 if acc else Src0,
                                   reference=ref), built)
        dve_ops_mod.OPS.append(op)
        dve_ops_mod._SUB_OPCODE_FOR_NAME[name] = row
        dve_ops_mod.CUSTOM_DVE_SPECS[name] = op.spec
        out.append(op)
    return out


# --------------------------------------------------------------------------
# Bass program
# --------------------------------------------------------------------------


def _build(compute=COMPUTE, h=H, r=R, reps=1, hbufs=HBUFS, ybufs=2):
    """Build the Bass program for one core: x [P,h+2,W+2] -> o [P,h,W]."""
    import concourse.bacc as bacc
    import concourse.mybir as mybir
    from concourse.tile import TileContext

    hdil3, hdil3acc = _get_hdil_ops()
    dt = mybir.dt.float16 if compute == "f16" else mybir.dt.float32
    f32 = mybir.dt.float32

    nc = bacc.Bacc()
    x_d = nc.dram_tensor("x", [P, h + 2, W + 2], dt, kind="ExternalInput")
    # se2[:, 3p+0] = se[c,p,1]; 3p+1 = se[c,p,2]-se[c,p,1]; 3p+2 = se[c,p,0]-se[c,p,1]
    se_d = nc.dram_tensor("se", [P, K * K], f32, kind="ExternalInput")
    o_d = nc.dram_tensor("o", [P, h, W], dt, kind="ExternalOutput")

    with TileContext(nc) as tc:
        with (
            tc.tile_pool(name="cpool", bufs=1) as cpool,
            tc.tile_pool(name="xpool", bufs=2) as xpool,
            tc.tile_pool(name="ypool", bufs=2) as ypool,
            tc.tile_pool(name="apool", bufs=2) as apool,
        ):
            se_sb = cpool.tile([P, K * K], f32)
            nc.sync.dma_start(out=se_sb[:], in_=se_d[:, :])

            L = r * (W + 2)
            for r0 in [v for _ in range(reps) for v in range(0, h, r)]:
                xt = xpool.tile([P, r + 2, W + 2], dt, tag="xt")
                nc.sync.dma_start(out=xt[:], in_=x_d[:, r0 : r0 + r + 2, :])

                ys = []
                for p in range(3):
                    y = ypool.tile([P, r, W + 2], dt, tag=f"y{p}", bufs=ybufs)
                    nc.scalar.add(y[:], xt[:, p : p + r, :],
                                  se_sb[:, 3 * p : 3 * p + 1])
                    ys.append(y)

                def flat(tile):
                    return tile[:].rearrange("p h w -> p (h w)")

                acc = None
                for p in range(3):
                    nxt = apool.tile([P, r, W + 2], dt, tag=f"a{p}",
                                     bufs=2 if p == 2 else hbufs)
                    kw = dict(
                        out=flat(nxt), in0=flat(ys[p]),
                        s0=se_sb[:, 3 * p + 1 : 3 * p + 2],
                        s1=se_sb[:, 3 * p + 2 : 3 * p + 3])
                    if acc is None:
                        nc.vector._custom_dve(hdil3, **kw)
                    else:
                        nc.vector._custom_dve(hdil3acc, in1=flat(acc), **kw)
                    acc = nxt

                nc.sync.dma_start(out=o_d[:, r0 : r0 + r, :],
                                  in_=acc[:, :, 2 : W + 2])
    nc.finalize()
    return nc


def _get_prog(key=("default",)):
    if key not in _prog_cache:
        _prog_cache[key] = _build()
    return _prog_cache[key]


def _pad_shard(x_shard, np_dt):
    """[BPC,C,H,W] fp32 -> zero-padded [P, H+2, W+2] in np_dt."""
    xp = np.zeros((P, HP, WP), np_dt)
    xp[:, 1 : H + 1, 1 : W + 1] = x_shard.reshape(P, H, W)
    return xp


def _prep_se(se):
    """se [C,3,3] fp32 -> derived per-partition consts [P, 9] fp32."""
    se = np.asarray(se, np.float32)
    d = np.empty((C, K * K), np.float32)
    for p in range(K):
        d[:, 3 * p + 0] = se[:, p, 1]
        d[:, 3 * p + 1] = se[:, p, 2] - se[:, p, 1]
        d[:, 3 * p + 2] = se[:, p, 0] - se[:, p, 1]
    return np.tile(d, (BPC, 1))


def _run(x, se, **spmd_kwargs):
    from concourse.bass_utils import run_bass_kernel_spmd

    nc = _get_prog()
    np_dt = np.float16 if COMPUTE == "f16" else np.float32
    x = np.asarray(x)
    se_p = _prep_se(se)
    in_maps = [
        {"x": _pad_shard(x[k * BPC : (k + 1) * BPC], np_dt), "se": se_p}
        for k in range(NCORES)
    ]
    res = run_bass_kernel_spmd(nc, in_maps, core_ids=list(range(NCORES)), **spmd_kwargs)
    out = np.empty((B, C, H, W), np.float32)
    for k in range(NCORES):
        out[k * BPC : (k + 1) * BPC] = (
            res.results[k]["o"].astype(np.float32).reshape(BPC, C, H, W)
        )
    return out, res


def kernel(x: np.ndarray, se: np.ndarray) -> np.ndarray:
    return _run(x, se)[0]


# revision 10
# speedup vs baseline: 1.8843x; 1.1979x over previous
"""Depthwise morphological (max-plus) dilation, 3x3, stride 1, zero-pad 1.

out[b,c,i,j] = max_{p,q} ( x_pad[b,c,i+p,j+q] + se[c,p,q] )

Sharding: pure data parallel over batch (16 batches -> 8 cores x 2).
On-core layout: partition dim = 2 batches x 64 channels = 128 planes;
each partition processes its own plane in row-blocks of R output rows.
The host supplies x zero-padded to [P, H+2, W+2] in fp16 plus a derived
per-partition SE tensor, so the device does no zero-fill.

Engine schedule — hand-written custom DVE microcode (3 uops/elem
instead of 17 stock ALU ops/elem):
  ACT   y_p = x + se[c,p,1]          (3 bias adds, rows p=0,1,2)
  DVE   h   = HDIL3(y_0)             out[k]=max(y[k]+d02, y[k-1], y[k-2]+d00)
  DVE   t   = HDIL3ACC(y_1, h)       ... max(..., h[k])
  DVE   o   = HDIL3ACC(y_2, t)
HDIL3* are custom DVE ops (registered into concourse.dve_ops at import):
the 3-tap sliding window lives in per-stage swap-flop delays (BYPASS
outputs operand a = the flop's previous value while capturing operand
b), so each op streams at 1 elem/cycle — the middle tap's constant is
folded into the ACT pre-add, the outer taps' deltas ride the two
per-partition scalar slots. DVE: 3 cyc/elem (~205us/core), ACT: 3
ops/elem (~180us/core), DMA ~95us/core — vs ~325us/core for the best
stock-op split (DVE 4x/2x perf-mode add/max chain).

fp16 compute gives ~2e-3 max abs err vs the fp32 reference (scale ~6).
"""

import numpy as np

B, C, H, W = 16, 64, 256, 256
K = 3
NCORES = 8
BPC = B // NCORES          # batches per core
P = BPC * C                # 128 partitions
HP, WP = H + 2, W + 2      # host-padded plane

COMPUTE = "f16"            # "f16" (fast, ~2e-3 abs err) or "f32" (exact)
R = 32                     # output rows per block
HBUFS = 1                  # bufs for the h/t intermediate tiles

_prog_cache = {}

# --------------------------------------------------------------------------
# Custom DVE ops: 3-tap sliding-window max-plus via swap-flop delays.
#   HDIL3_ANT:    out[k] = max(y[k]+C0, y[k-1], y[k-2]+C1)
#   HDIL3ACC_ANT: out[k] = max(y[k]+C0, y[k-1], y[k-2]+C1, z[k])
# C0/C1 are per-partition scalars (s0/s1 APs). First two outputs of each
# instruction are stale-flop garbage; the stream layout keeps those in
# never-stored junk columns.
# --------------------------------------------------------------------------


class _HandDveOp:
    """Duck-typed stand-in for dve_ops.DveOp with hand-written uops."""

    def __init__(self, name, spec, built, subdim=False):
        self.name, self.spec, self.subdim = name, spec, subdim
        self._built = built

    def compile(self, ver):
        assert ver == "v3", f"hand op {self.name} only built for v3, got {ver}"
        return self._built


def _mk_hdil3_uop(acc):
    from concourse.dve_uop import (
        UopConfig, AluOp, AluInp, InpSel, OutSel, OutPath, Trigger,
        DelayInp, ENABLE,
    )

    u = UopConfig()
    u.enable_input(InpSel.SRC_0, 1)    # chain0 = y[k]
    u.enable_input(InpSel.CONST_0, 2)  # chain1 = C0 (tap k   / SE col 2)
    u.enable_input(InpSel.CONST_1, 3)  # chain2 = C1 (tap k-2 / SE col 0)
    if acc:
        u.enable_input(InpSel.SRC_1, 4)  # chain3 = z[k]
        u.require_inp1 = ENABLE
    u.require_inp0 = ENABLE
    u.trigger = (Trigger.SRC_TENSOR_DONE, Trigger.NONE, Trigger.NONE)
    dp = u.datapath_config

    for st in range(8):
        dp[st].pass_through_delay(*([0, 1, 2] + ([3] if acc else [])))
    dly = 4 if acc else 3   # chain carrying y[k-1]
    mch = 5 if acc else 4   # chain carrying m0

    def delay_block(blk, src1):
        # out = swap_prev; swap <- operand b  (1-element delay)
        blk.op = AluOp.BYPASS
        blk.alu_src0 = AluInp.CURR_SWAP_OUT
        blk.alu_src1 = src1
        blk.alu_out_enable = ENABLE
        blk.swap_enable = ENABLE

    delay_block(dp[0], AluInp.PREV_DELAY_0)                  # y[k-1]
    delay_block(dp[1], AluInp.PREV_ALU_OUT)                  # y[k-2]
    dp[1].enable_delay_from_src(DelayInp.PREV_ALU_OUT, dly)  # chain <- y[k-1]
    dp[2].enable_alu(AluOp.ADD, AluInp.PREV_ALU_OUT, AluInp.PREV_DELAY_2)
    dp[2].pass_through_delay(dly)
    dp[3].enable_alu(AluOp.MAX, AluInp.PREV_ALU_OUT,
                     AluInp(AluInp.PREV_DELAY_0 + dly))      # max(., y[k-1])
    dp[4].enable_alu(AluOp.ADD, AluInp.PREV_DELAY_0, AluInp.PREV_DELAY_1)
    dp[4].enable_delay_from_src(DelayInp.PREV_ALU_OUT, mch)  # chain <- m0
    dp[5].enable_alu(AluOp.MAX, AluInp.PREV_ALU_OUT,
                     AluInp(AluInp.PREV_DELAY_0 + mch))      # max(y[k]+C0, m0)
    if acc:
        dp[6].enable_alu(AluOp.MAX, AluInp.PREV_ALU_OUT, AluInp.PREV_DELAY_3)
    else:
        dp[6].pass_through_alu()
    dp[7].pass_through_alu()
    u.enable_output(OutSel.ALU_OUT, OutPath.WR0_LO)
    return u


def _hdil3_core(y, c0, c1):
    out = np.empty_like(y)
    out[..., 2:] = np.maximum(
        np.maximum(y[..., 2:] + c0, y[..., 1:-1]), y[..., :-2] + c1)
    out[..., :2] = 0.0  # HW: stale flop garbage
    return out


def _get_hdil_ops():
    import concourse.dve_ops as dve_ops_mod
    from concourse.dve_spec import Spec, Src0, Src1
    from concourse.dve_uop import DveOpSpec

    def ref1(in0, in1, s0, s1, imm2):
        return _hdil3_core(in0, np.asarray(s0)[..., None],
                           np.asarray(s1)[..., None])

    def ref2(in0, in1, s0, s1, imm2):
        r = ref1(in0, None, s0, s1, imm2)
        r[..., 2:] = np.maximum(r[..., 2:], in1[..., 2:])
        return r

    out = []
    for name, acc, ref in (("HDIL3_ANT", False, ref1),
                           ("HDIL3ACC_ANT", True, ref2)):
        if name in dve_ops_mod._SUB_OPCODE_FOR_NAME:
            out.append(next(o for o in dve_ops_mod.OPS if o.name == name))
            continue
        row = max(dve_ops_mod._SUB_OPCODE_FOR_NAME.values()) + 1
        assert row < 0x20
        built = DveOpSpec(name=name, uops=[_mk_hdil3_uop(acc)],
                          opcode=row, rd1_en=acc)
        built.validate("v3")
        op = _HandDveOp(name, Spec(body=Src0 + Src1 if acc else Src0,
                                   reference=ref), built)
        dve_ops_mod.OPS.append(op)
        dve_ops_mod._SUB_OPCODE_FOR_NAME[name] = row
        dve_ops_mod.CUSTOM_DVE_SPECS[name] = op.spec
        out.append(op)
    return out




# --------------------------------------------------------------------------
# v3: 2x-perf-mode ops.
#   HDIL2X_ANT: out[k] = max(a[k]+C0, y[k-1], y[k-2]+C1) with a = y[2:],
#     b = y[:-2] passed as the two streams (shift-view, no cross-cycle
#     state) -> fits a 2-elems/cycle uop program (8 ALU blocks / cycle).
#   PMAX2C_ANT: out[k] = max(a[k]+C0, b[k]+C1), 2 elems/cycle.
# Emitted instructions get .ins.perf_max = 1 so the engine engages the
# 2X_1PORT slot (fp16, stride-1, 4B-aligned streams guaranteed below).
# --------------------------------------------------------------------------


def _mk_v3_uops():
    from concourse.dve_uop import (
        UopConfig, AluOp, AluInp, InpSel, OutSel, OutPath, Trigger,
        DelayInp, ENABLE,
    )
    D = AluInp.PREV_DELAY_0

    def base(lanes):
        u = UopConfig()
        for src, lane in lanes:
            u.enable_input(src, lane)
        u.require_inp0 = ENABLE
        u.require_inp1 = ENABLE
        u.trigger = (Trigger.SRC_TENSOR_DONE, Trigger.NONE, Trigger.NONE)
        return u

    two_src = [(InpSel.SRC_0, 1), (InpSel.SRC_0_HI, 2), (InpSel.SRC_1, 3),
               (InpSel.SRC_1_HI, 4), (InpSel.CONST_0, 5), (InpSel.CONST_1, 6)]
    one_src = [(InpSel.SRC_0, 1), (InpSel.SRC_1, 3),
               (InpSel.CONST_0, 5), (InpSel.CONST_1, 6)]

    # HDIL2X 2x: c0=y[k] c1=y[k+1] c2=y[k-2] c3=y[k-1] c4=C0 c5=C1
    u = base(two_src)
    dp = u.datapath_config
    dp[0].enable_alu(AluOp.ADD, AluInp(D + 3), AluInp(D + 5))
    dp[0].pass_through_delay(0, 1, 2, 3, 4, 5)
    dp[1].enable_alu(AluOp.ADD, AluInp(D + 2), AluInp(D + 5))
    dp[1].pass_through_delay(0, 1, 3, 4)
    dp[1].enable_delay_from_src(DelayInp.PREV_ALU_OUT, 2)   # c2 <- a4
    dp[2].enable_alu(AluOp.MAX, AluInp.PREV_ALU_OUT, AluInp(D + 3))
    dp[2].pass_through_delay(0, 1, 2, 4)
    dp[3].enable_alu(AluOp.ADD, AluInp(D + 0), AluInp(D + 4))
    dp[3].pass_through_delay(0, 1, 2, 4)
    dp[3].enable_delay_from_src(DelayInp.PREV_ALU_OUT, 3)   # c3 <- m1
    dp[4].enable_alu(AluOp.MAX, AluInp.PREV_ALU_OUT, AluInp(D + 3))
    dp[4].pass_through_delay(0, 1, 2, 4)
    dp[5].enable_alu(AluOp.ADD, AluInp(D + 1), AluInp(D + 4))
    dp[5].pass_through_delay(0, 2)
    dp[5].enable_delay_from_src(DelayInp.PREV_ALU_OUT, 1)   # c1 <- OUT_k
    dp[6].enable_alu(AluOp.MAX, AluInp(D + 2), AluInp(D + 0))
    dp[6].pass_through_delay(1)
    dp[6].enable_delay_from_src(DelayInp.PREV_ALU_OUT, 2)   # c2 <- a3
    dp[7].enable_alu(AluOp.MAX, AluInp.PREV_ALU_OUT, AluInp(D + 2))
    dp[7].pass_through_delay(1)
    u.enable_output(OutSel.DELAY_1, OutPath.WR0_LO)
    u.enable_output(OutSel.ALU_OUT, OutPath.WR0_HI)
    hdil_2x = u

    # HDIL2X 1x fallback (first output stale)
    u = base(one_src)
    dp = u.datapath_config
    dp[0].op = AluOp.BYPASS
    dp[0].alu_src0 = AluInp.CURR_SWAP_OUT
    dp[0].alu_src1 = AluInp(D + 0)
    dp[0].alu_out_enable = ENABLE
    dp[0].swap_enable = ENABLE
    dp[0].pass_through_delay(0, 2, 4, 5)
    dp[1].enable_alu(AluOp.ADD, AluInp(D + 2), AluInp(D + 5))
    dp[1].pass_through_delay(0, 4)
    dp[1].enable_delay_from_src(DelayInp.PREV_ALU_OUT, 2)
    dp[2].enable_alu(AluOp.MAX, AluInp.PREV_ALU_OUT, AluInp(D + 2))
    dp[2].pass_through_delay(0, 4)
    dp[3].enable_alu(AluOp.ADD, AluInp(D + 0), AluInp(D + 4))
    dp[3].enable_delay_from_src(DelayInp.PREV_ALU_OUT, 2)
    dp[4].enable_alu(AluOp.MAX, AluInp.PREV_ALU_OUT, AluInp(D + 2))
    for st in (5, 6, 7):
        dp[st].pass_through_alu()
    u.enable_output(OutSel.ALU_OUT, OutPath.WR0_LO)
    hdil_1x = u

    # PMAX2C 2x
    u = base(two_src)
    dp = u.datapath_config
    dp[0].enable_alu(AluOp.ADD, AluInp(D + 0), AluInp(D + 4))
    dp[0].pass_through_delay(1, 2, 3, 4, 5)
    dp[1].enable_alu(AluOp.ADD, AluInp(D + 2), AluInp(D + 5))
    dp[1].pass_through_delay(1, 3, 4, 5)
    dp[1].enable_delay_from_src(DelayInp.PREV_ALU_OUT, 0)   # c0 <- p0
    dp[2].enable_alu(AluOp.MAX, AluInp.PREV_ALU_OUT, AluInp(D + 0))
    dp[2].pass_through_delay(1, 3, 4, 5)
    dp[3].enable_alu(AluOp.ADD, AluInp(D + 1), AluInp(D + 4))
    dp[3].pass_through_delay(3, 5)
    dp[3].enable_delay_from_src(DelayInp.PREV_ALU_OUT, 0)   # c0 <- OUT_k
    dp[4].enable_alu(AluOp.ADD, AluInp(D + 3), AluInp(D + 5))
    dp[4].pass_through_delay(0)
    dp[4].enable_delay_from_src(DelayInp.PREV_ALU_OUT, 1)   # c1 <- p1
    dp[5].enable_alu(AluOp.MAX, AluInp.PREV_ALU_OUT, AluInp(D + 1))
    dp[5].pass_through_delay(0)
    for st in (6, 7):
        dp[st].pass_through_alu()
        dp[st].pass_through_delay(0)
    u.enable_output(OutSel.DELAY_0, OutPath.WR0_LO)
    u.enable_output(OutSel.ALU_OUT, OutPath.WR0_HI)
    pmax_2x = u

    # PMAX2C 1x fallback (exact)
    u = base(one_src)
    dp = u.datapath_config
    dp[0].enable_alu(AluOp.ADD, AluInp(D + 0), AluInp(D + 4))
    dp[0].pass_through_delay(2, 5)
    dp[1].enable_alu(AluOp.ADD, AluInp(D + 2), AluInp(D + 5))
    dp[1].enable_delay_from_src(DelayInp.PREV_ALU_OUT, 0)
    dp[2].enable_alu(AluOp.MAX, AluInp.PREV_ALU_OUT, AluInp(D + 0))
    for st in (3, 4, 5, 6, 7):
        dp[st].pass_through_alu()
    u.enable_output(OutSel.ALU_OUT, OutPath.WR0_LO)
    pmax_1x = u

    return hdil_1x, hdil_2x, pmax_1x, pmax_2x


def _hdil2x_ref(in0, in1, s0, s1, imm2):
    c0 = np.asarray(s0)[..., None]
    c1 = np.asarray(s1)[..., None]
    mid = np.empty_like(in0)
    mid[..., :-1] = in1[..., 1:]
    mid[..., -1] = in0[..., -2]
    return np.maximum(np.maximum(in0 + c0, mid), in1 + c1)


def _pmax2c_ref(in0, in1, s0, s1, imm2):
    return np.maximum(in0 + np.asarray(s0)[..., None],
                      in1 + np.asarray(s1)[..., None])


def _get_v3_ops():
    import concourse.dve_ops as dve_ops_mod
    from concourse.dve_spec import Spec, Src0, Src1
    from concourse.dve_uop import DveOpSpec

    hdil_1x, hdil_2x, pmax_1x, pmax_2x = _mk_v3_uops()
    out = []
    for name, u1, u2, ref in (("HDIL2X_ANT", hdil_1x, hdil_2x, _hdil2x_ref),
                              ("PMAX2C_ANT", pmax_1x, pmax_2x, _pmax2c_ref)):
        if name in dve_ops_mod._SUB_OPCODE_FOR_NAME:
            out.append(next(o for o in dve_ops_mod.OPS if o.name == name))
            continue
        row = max(dve_ops_mod._SUB_OPCODE_FOR_NAME.values()) + 1
        assert row < 0x20
        built = DveOpSpec(name=name, uops=[u1], uops_2x=[u2],
                          opcode=row, rd1_en=True, perf_max=1)
        built.validate("v3")
        op = _HandDveOp(name, Spec(body=Src0 + Src1, reference=ref), built)
        dve_ops_mod.OPS.append(op)
        dve_ops_mod._SUB_OPCODE_FOR_NAME[name] = row
        dve_ops_mod.CUSTOM_DVE_SPECS[name] = op.spec
        out.append(op)
    return out


# --------------------------------------------------------------------------
# Bass program
# --------------------------------------------------------------------------


def _build(compute=COMPUTE, h=H, r=R, reps=1, hbufs=HBUFS, ybufs=2):
    """Build the Bass program for one core: x [P,h+2,W+2] -> o [P,h,W].

    v3: 5 DVE instructions per block, all 2 elems/cycle:
      W_p = HDIL2X(xt rows p..p+r-1)            p = 0,1,2
      M   = PMAX2C(W_0 + c01, W_1 + c11)
      o   = PMAX2C(M + 0,     W_2 + c21)
    ACT/PE/GPSIMD idle; DVE ~2.5 cyc/elem.
    """
    import concourse.bacc as bacc
    import concourse.mybir as mybir
    from concourse.tile import TileContext

    hdil2x, pmax2c = _get_v3_ops()
    dt = mybir.dt.float16 if compute == "f16" else mybir.dt.float32
    f32 = mybir.dt.float32

    nc = bacc.Bacc()
    x_d = nc.dram_tensor("x", [P, h + 2, W + 2], dt, kind="ExternalInput")
    # se2[:, 3p+0] = se[c,p,1]; 3p+1 = se[c,p,2]-se[c,p,1]; 3p+2 = se[c,p,0]-se[c,p,1]
    se_d = nc.dram_tensor("se", [P, K * K], f32, kind="ExternalInput")
    o_d = nc.dram_tensor("o", [P, h, W], dt, kind="ExternalOutput")

    WP_ = W + 2

    def pm(inst):
        getattr(inst, "ins", inst).perf_max = 1
        return inst

    with TileContext(nc) as tc:
        with (
            tc.tile_pool(name="cpool", bufs=1) as cpool,
            tc.tile_pool(name="xpool", bufs=2) as xpool,
            tc.tile_pool(name="wpool", bufs=hbufs) as wpool,
            tc.tile_pool(name="opool", bufs=2) as opool,
        ):
            se_sb = cpool.tile([P, K * K], f32)
            nc.sync.dma_start(out=se_sb[:], in_=se_d[:, :])

            L = r * WP_
            for r0 in [v for _ in range(reps) for v in range(0, h, r)]:
                xt = xpool.tile([P, r + 2, WP_], dt, tag="xt")
                nc.sync.dma_start(out=xt[:], in_=x_d[:, r0 : r0 + r + 2, :])
                xf = xt[:].rearrange("p h w -> p (h w)")

                def flat(tile):
                    return tile[:].rearrange("p h w -> p (h w)")

                ws = []
                for p in range(3):
                    wt = wpool.tile([P, r, WP_], dt, tag=f"w{p}")
                    off = p * WP_
                    pm(nc.vector._custom_dve(
                        hdil2x,
                        out=flat(wt)[:, 2:L],
                        in0=xf[:, off + 2 : off + L],
                        in1=xf[:, off : off + L - 2],
                        s0=se_sb[:, 3 * p + 1 : 3 * p + 2],
                        s1=se_sb[:, 3 * p + 2 : 3 * p + 3]))
                    ws.append(wt)

                mt = wpool.tile([P, r, WP_], dt, tag="m")
                pm(nc.vector._custom_dve(
                    pmax2c, out=flat(mt), in0=flat(ws[0]), in1=flat(ws[1]),
                    s0=se_sb[:, 0:1], s1=se_sb[:, 3:4]))
                ot = opool.tile([P, r, WP_], dt, tag="o")
                pm(nc.vector._custom_dve(
                    pmax2c, out=flat(ot), in0=flat(mt), in1=flat(ws[2]),
                    s0=0.0, s1=se_sb[:, 6:7]))

                nc.sync.dma_start(out=o_d[:, r0 : r0 + r, :],
                                  in_=ot[:, :, 2 : W + 2])
    nc.finalize()
    return nc


def _get_prog(key=("default",)):
    if key not in _prog_cache:
        _prog_cache[key] = _build()
    return _prog_cache[key]


def _pad_shard(x_shard, np_dt):
    """[BPC,C,H,W] fp32 -> zero-padded [P, H+2, W+2] in np_dt."""
    xp = np.zeros((P, HP, WP), np_dt)
    xp[:, 1 : H + 1, 1 : W + 1] = x_shard.reshape(P, H, W)
    return xp


def _prep_se(se):
    """se [C,3,3] fp32 -> derived per-partition consts [P, 9] fp32."""
    se = np.asarray(se, np.float32)
    d = np.empty((C, K * K), np.float32)
    for p in range(K):
        d[:, 3 * p + 0] = se[:, p, 1]
        d[:, 3 * p + 1] = se[:, p, 2] - se[:, p, 1]
        d[:, 3 * p + 2] = se[:, p, 0] - se[:, p, 1]
    return np.tile(d, (BPC, 1))


def _run(x, se, **spmd_kwargs):
    from concourse.bass_utils import run_bass_kernel_spmd

    nc = _get_prog()
    np_dt = np.float16 if COMPUTE == "f16" else np.float32
    x = np.asarray(x)
    se_p = _prep_se(se)
    in_maps = [
        {"x": _pad_shard(x[k * BPC : (k + 1) * BPC], np_dt), "se": se_p}
        for k in range(NCORES)
    ]
    res = run_bass_kernel_spmd(nc, in_maps, core_ids=list(range(NCORES)), **spmd_kwargs)
    out = np.empty((B, C, H, W), np.float32)
    for k in range(NCORES):
        out[k * BPC : (k + 1) * BPC] = (
            res.results[k]["o"].astype(np.float32).reshape(BPC, C, H, W)
        )
    return out, res


def kernel(x: np.ndarray, se: np.ndarray) -> np.ndarray:
    return _run(x, se)[0]


# revision 11
# speedup vs baseline: 1.9036x; 1.0103x over previous
"""Depthwise morphological (max-plus) dilation, 3x3, stride 1, zero-pad 1.

out[b,c,i,j] = max_{p,q} ( x_pad[b,c,i+p,j+q] + se[c,p,q] )

Sharding: pure data parallel over batch (16 batches -> 8 cores x 2).
On-core layout: partition dim = 2 batches x 64 channels = 128 planes;
each partition processes its own plane in row-blocks of R output rows.
The host supplies x zero-padded to [P, H+2, W+2] in fp16 plus a derived
per-partition SE tensor, so the device does no zero-fill.

Engine schedule — hand-written custom DVE microcode (3 uops/elem
instead of 17 stock ALU ops/elem):
  ACT   y_p = x + se[c,p,1]          (3 bias adds, rows p=0,1,2)
  DVE   h   = HDIL3(y_0)             out[k]=max(y[k]+d02, y[k-1], y[k-2]+d00)
  DVE   t   = HDIL3ACC(y_1, h)       ... max(..., h[k])
  DVE   o   = HDIL3ACC(y_2, t)
HDIL3* are custom DVE ops (registered into concourse.dve_ops at import):
the 3-tap sliding window lives in per-stage swap-flop delays (BYPASS
outputs operand a = the flop's previous value while capturing operand
b), so each op streams at 1 elem/cycle — the middle tap's constant is
folded into the ACT pre-add, the outer taps' deltas ride the two
per-partition scalar slots. DVE: 3 cyc/elem (~205us/core), ACT: 3
ops/elem (~180us/core), DMA ~95us/core — vs ~325us/core for the best
stock-op split (DVE 4x/2x perf-mode add/max chain).

fp16 compute gives ~2e-3 max abs err vs the fp32 reference (scale ~6).
"""

import numpy as np

B, C, H, W = 16, 64, 256, 256
K = 3
NCORES = 8
BPC = B // NCORES          # batches per core
P = BPC * C                # 128 partitions
HP, WP = H + 2, W + 2      # host-padded plane

COMPUTE = "f16"            # "f16" (fast, ~2e-3 abs err) or "f32" (exact)
R = 32                     # output rows per block
HBUFS = 2                  # bufs for the W/M intermediate tiles

_prog_cache = {}

# --------------------------------------------------------------------------
# Custom DVE ops: 3-tap sliding-window max-plus via swap-flop delays.
#   HDIL3_ANT:    out[k] = max(y[k]+C0, y[k-1], y[k-2]+C1)
#   HDIL3ACC_ANT: out[k] = max(y[k]+C0, y[k-1], y[k-2]+C1, z[k])
# C0/C1 are per-partition scalars (s0/s1 APs). First two outputs of each
# instruction are stale-flop garbage; the stream layout keeps those in
# never-stored junk columns.
# --------------------------------------------------------------------------


class _HandDveOp:
    """Duck-typed stand-in for dve_ops.DveOp with hand-written uops."""

    def __init__(self, name, spec, built, subdim=False):
        self.name, self.spec, self.subdim = name, spec, subdim
        self._built = built

    def compile(self, ver):
        assert ver == "v3", f"hand op {self.name} only built for v3, got {ver}"
        return self._built


def _mk_hdil3_uop(acc):
    from concourse.dve_uop import (
        UopConfig, AluOp, AluInp, InpSel, OutSel, OutPath, Trigger,
        DelayInp, ENABLE,
    )

    u = UopConfig()
    u.enable_input(InpSel.SRC_0, 1)    # chain0 = y[k]
    u.enable_input(InpSel.CONST_0, 2)  # chain1 = C0 (tap k   / SE col 2)
    u.enable_input(InpSel.CONST_1, 3)  # chain2 = C1 (tap k-2 / SE col 0)
    if acc:
        u.enable_input(InpSel.SRC_1, 4)  # chain3 = z[k]
        u.require_inp1 = ENABLE
    u.require_inp0 = ENABLE
    u.trigger = (Trigger.SRC_TENSOR_DONE, Trigger.NONE, Trigger.NONE)
    dp = u.datapath_config

    for st in range(8):
        dp[st].pass_through_delay(*([0, 1, 2] + ([3] if acc else [])))
    dly = 4 if acc else 3   # chain carrying y[k-1]
    mch = 5 if acc else 4   # chain carrying m0

    def delay_block(blk, src1):
        # out = swap_prev; swap <- operand b  (1-element delay)
        blk.op = AluOp.BYPASS
        blk.alu_src0 = AluInp.CURR_SWAP_OUT
        blk.alu_src1 = src1
        blk.alu_out_enable = ENABLE
        blk.swap_enable = ENABLE

    delay_block(dp[0], AluInp.PREV_DELAY_0)                  # y[k-1]
    delay_block(dp[1], AluInp.PREV_ALU_OUT)                  # y[k-2]
    dp[1].enable_delay_from_src(DelayInp.PREV_ALU_OUT, dly)  # chain <- y[k-1]
    dp[2].enable_alu(AluOp.ADD, AluInp.PREV_ALU_OUT, AluInp.PREV_DELAY_2)
    dp[2].pass_through_delay(dly)
    dp[3].enable_alu(AluOp.MAX, AluInp.PREV_ALU_OUT,
                     AluInp(AluInp.PREV_DELAY_0 + dly))      # max(., y[k-1])
    dp[4].enable_alu(AluOp.ADD, AluInp.PREV_DELAY_0, AluInp.PREV_DELAY_1)
    dp[4].enable_delay_from_src(DelayInp.PREV_ALU_OUT, mch)  # chain <- m0
    dp[5].enable_alu(AluOp.MAX, AluInp.PREV_ALU_OUT,
                     AluInp(AluInp.PREV_DELAY_0 + mch))      # max(y[k]+C0, m0)
    if acc:
        dp[6].enable_alu(AluOp.MAX, AluInp.PREV_ALU_OUT, AluInp.PREV_DELAY_3)
    else:
        dp[6].pass_through_alu()
    dp[7].pass_through_alu()
    u.enable_output(OutSel.ALU_OUT, OutPath.WR0_LO)
    return u


def _hdil3_core(y, c0, c1):
    out = np.empty_like(y)
    out[..., 2:] = np.maximum(
        np.maximum(y[..., 2:] + c0, y[..., 1:-1]), y[..., :-2] + c1)
    out[..., :2] = 0.0  # HW: stale flop garbage
    return out


def _get_hdil_ops():
    import concourse.dve_ops as dve_ops_mod
    from concourse.dve_spec import Spec, Src0, Src1
    from concourse.dve_uop import DveOpSpec

    def ref1(in0, in1, s0, s1, imm2):
        return _hdil3_core(in0, np.asarray(s0)[..., None],
                           np.asarray(s1)[..., None])

    def ref2(in0, in1, s0, s1, imm2):
        r = ref1(in0, None, s0, s1, imm2)
        r[..., 2:] = np.maximum(r[..., 2:], in1[..., 2:])
        return r

    out = []
    for name, acc, ref in (("HDIL3_ANT", False, ref1),
                           ("HDIL3ACC_ANT", True, ref2)):
        if name in dve_ops_mod._SUB_OPCODE_FOR_NAME:
            out.append(next(o for o in dve_ops_mod.OPS if o.name == name))
            continue
        row = max(dve_ops_mod._SUB_OPCODE_FOR_NAME.values()) + 1
        assert row < 0x20
        built = DveOpSpec(name=name, uops=[_mk_hdil3_uop(acc)],
                          opcode=row, rd1_en=acc)
        built.validate("v3")
        op = _HandDveOp(name, Spec(body=Src0 + Src1 if acc else Src0,
                                   reference=ref), built)
        dve_ops_mod.OPS.append(op)
        dve_ops_mod._SUB_OPCODE_FOR_NAME[name] = row
        dve_ops_mod.CUSTOM_DVE_SPECS[name] = op.spec
        out.append(op)
    return out




# --------------------------------------------------------------------------
# v3: 2x-perf-mode ops.
#   HDIL2X_ANT: out[k] = max(a[k]+C0, y[k-1], y[k-2]+C1) with a = y[2:],
#     b = y[:-2] passed as the two streams (shift-view, no cross-cycle
#     state) -> fits a 2-elems/cycle uop program (8 ALU blocks / cycle).
#   PMAX2C_ANT: out[k] = max(a[k]+C0, b[k]+C1), 2 elems/cycle.
# Emitted instructions get .ins.perf_max = 1 so the engine engages the
# 2X_1PORT slot (fp16, stride-1, 4B-aligned streams guaranteed below).
# --------------------------------------------------------------------------


def _mk_v3_uops():
    from concourse.dve_uop import (
        UopConfig, AluOp, AluInp, InpSel, OutSel, OutPath, Trigger,
        DelayInp, ENABLE,
    )
    D = AluInp.PREV_DELAY_0

    def base(lanes):
        u = UopConfig()
        for src, lane in lanes:
            u.enable_input(src, lane)
        u.require_inp0 = ENABLE
        u.require_inp1 = ENABLE
        u.trigger = (Trigger.SRC_TENSOR_DONE, Trigger.NONE, Trigger.NONE)
        return u

    two_src = [(InpSel.SRC_0, 1), (InpSel.SRC_0_HI, 2), (InpSel.SRC_1, 3),
               (InpSel.SRC_1_HI, 4), (InpSel.CONST_0, 5), (InpSel.CONST_1, 6)]
    one_src = [(InpSel.SRC_0, 1), (InpSel.SRC_1, 3),
               (InpSel.CONST_0, 5), (InpSel.CONST_1, 6)]

    # HDIL2X 2x: c0=y[k] c1=y[k+1] c2=y[k-2] c3=y[k-1] c4=C0 c5=C1
    u = base(two_src)
    dp = u.datapath_config
    dp[0].enable_alu(AluOp.ADD, AluInp(D + 3), AluInp(D + 5))
    dp[0].pass_through_delay(0, 1, 2, 3, 4, 5)
    dp[1].enable_alu(AluOp.ADD, AluInp(D + 2), AluInp(D + 5))
    dp[1].pass_through_delay(0, 1, 3, 4)
    dp[1].enable_delay_from_src(DelayInp.PREV_ALU_OUT, 2)   # c2 <- a4
    dp[2].enable_alu(AluOp.MAX, AluInp.PREV_ALU_OUT, AluInp(D + 3))
    dp[2].pass_through_delay(0, 1, 2, 4)
    dp[3].enable_alu(AluOp.ADD, AluInp(D + 0), AluInp(D + 4))
    dp[3].pass_through_delay(0, 1, 2, 4)
    dp[3].enable_delay_from_src(DelayInp.PREV_ALU_OUT, 3)   # c3 <- m1
    dp[4].enable_alu(AluOp.MAX, AluInp.PREV_ALU_OUT, AluInp(D + 3))
    dp[4].pass_through_delay(0, 1, 2, 4)
    dp[5].enable_alu(AluOp.ADD, AluInp(D + 1), AluInp(D + 4))
    dp[5].pass_through_delay(0, 2)
    dp[5].enable_delay_from_src(DelayInp.PREV_ALU_OUT, 1)   # c1 <- OUT_k
    dp[6].enable_alu(AluOp.MAX, AluInp(D + 2), AluInp(D + 0))
    dp[6].pass_through_delay(1)
    dp[6].enable_delay_from_src(DelayInp.PREV_ALU_OUT, 2)   # c2 <- a3
    dp[7].enable_alu(AluOp.MAX, AluInp.PREV_ALU_OUT, AluInp(D + 2))
    dp[7].pass_through_delay(1)
    u.enable_output(OutSel.DELAY_1, OutPath.WR0_LO)
    u.enable_output(OutSel.ALU_OUT, OutPath.WR0_HI)
    hdil_2x = u

    # HDIL2X 1x fallback (first output stale)
    u = base(one_src)
    dp = u.datapath_config
    dp[0].op = AluOp.BYPASS
    dp[0].alu_src0 = AluInp.CURR_SWAP_OUT
    dp[0].alu_src1 = AluInp(D + 0)
    dp[0].alu_out_enable = ENABLE
    dp[0].swap_enable = ENABLE
    dp[0].pass_through_delay(0, 2, 4, 5)
    dp[1].enable_alu(AluOp.ADD, AluInp(D + 2), AluInp(D + 5))
    dp[1].pass_through_delay(0, 4)
    dp[1].enable_delay_from_src(DelayInp.PREV_ALU_OUT, 2)
    dp[2].enable_alu(AluOp.MAX, AluInp.PREV_ALU_OUT, AluInp(D + 2))
    dp[2].pass_through_delay(0, 4)
    dp[3].enable_alu(AluOp.ADD, AluInp(D + 0), AluInp(D + 4))
    dp[3].enable_delay_from_src(DelayInp.PREV_ALU_OUT, 2)
    dp[4].enable_alu(AluOp.MAX, AluInp.PREV_ALU_OUT, AluInp(D + 2))
    for st in (5, 6, 7):
        dp[st].pass_through_alu()
    u.enable_output(OutSel.ALU_OUT, OutPath.WR0_LO)
    hdil_1x = u

    # PMAX2C 2x
    u = base(two_src)
    dp = u.datapath_config
    dp[0].enable_alu(AluOp.ADD, AluInp(D + 0), AluInp(D + 4))
    dp[0].pass_through_delay(1, 2, 3, 4, 5)
    dp[1].enable_alu(AluOp.ADD, AluInp(D + 2), AluInp(D + 5))
    dp[1].pass_through_delay(1, 3, 4, 5)
    dp[1].enable_delay_from_src(DelayInp.PREV_ALU_OUT, 0)   # c0 <- p0
    dp[2].enable_alu(AluOp.MAX, AluInp.PREV_ALU_OUT, AluInp(D + 0))
    dp[2].pass_through_delay(1, 3, 4, 5)
    dp[3].enable_alu(AluOp.ADD, AluInp(D + 1), AluInp(D + 4))
    dp[3].pass_through_delay(3, 5)
    dp[3].enable_delay_from_src(DelayInp.PREV_ALU_OUT, 0)   # c0 <- OUT_k
    dp[4].enable_alu(AluOp.ADD, AluInp(D + 3), AluInp(D + 5))
    dp[4].pass_through_delay(0)
    dp[4].enable_delay_from_src(DelayInp.PREV_ALU_OUT, 1)   # c1 <- p1
    dp[5].enable_alu(AluOp.MAX, AluInp.PREV_ALU_OUT, AluInp(D + 1))
    dp[5].pass_through_delay(0)
    for st in (6, 7):
        dp[st].pass_through_alu()
        dp[st].pass_through_delay(0)
    u.enable_output(OutSel.DELAY_0, OutPath.WR0_LO)
    u.enable_output(OutSel.ALU_OUT, OutPath.WR0_HI)
    pmax_2x = u

    # PMAX2C 1x fallback (exact)
    u = base(one_src)
    dp = u.datapath_config
    dp[0].enable_alu(AluOp.ADD, AluInp(D + 0), AluInp(D + 4))
    dp[0].pass_through_delay(2, 5)
    dp[1].enable_alu(AluOp.ADD, AluInp(D + 2), AluInp(D + 5))
    dp[1].enable_delay_from_src(DelayInp.PREV_ALU_OUT, 0)
    dp[2].enable_alu(AluOp.MAX, AluInp.PREV_ALU_OUT, AluInp(D + 0))
    for st in (3, 4, 5, 6, 7):
        dp[st].pass_through_alu()
    u.enable_output(OutSel.ALU_OUT, OutPath.WR0_LO)
    pmax_1x = u

    return hdil_1x, hdil_2x, pmax_1x, pmax_2x


def _hdil2x_ref(in0, in1, s0, s1, imm2):
    c0 = np.asarray(s0)[..., None]
    c1 = np.asarray(s1)[..., None]
    mid = np.empty_like(in0)
    mid[..., :-1] = in1[..., 1:]
    mid[..., -1] = in0[..., -2]
    return np.maximum(np.maximum(in0 + c0, mid), in1 + c1)


def _pmax2c_ref(in0, in1, s0, s1, imm2):
    return np.maximum(in0 + np.asarray(s0)[..., None],
                      in1 + np.asarray(s1)[..., None])


def _get_v3_ops():
    import concourse.dve_ops as dve_ops_mod
    from concourse.dve_spec import Spec, Src0, Src1
    from concourse.dve_uop import DveOpSpec

    hdil_1x, hdil_2x, pmax_1x, pmax_2x = _mk_v3_uops()
    out = []
    for name, u1, u2, ref in (("HDIL2X_ANT", hdil_1x, hdil_2x, _hdil2x_ref),
                              ("PMAX2C_ANT", pmax_1x, pmax_2x, _pmax2c_ref)):
        if name in dve_ops_mod._SUB_OPCODE_FOR_NAME:
            out.append(next(o for o in dve_ops_mod.OPS if o.name == name))
            continue
        row = max(dve_ops_mod._SUB_OPCODE_FOR_NAME.values()) + 1
        assert row < 0x20
        built = DveOpSpec(name=name, uops=[u1], uops_2x=[u2],
                          opcode=row, rd1_en=True, perf_max=1)
        built.validate("v3")
        op = _HandDveOp(name, Spec(body=Src0 + Src1, reference=ref), built)
        dve_ops_mod.OPS.append(op)
        dve_ops_mod._SUB_OPCODE_FOR_NAME[name] = row
        dve_ops_mod.CUSTOM_DVE_SPECS[name] = op.spec
        out.append(op)
    return out


# --------------------------------------------------------------------------
# Bass program
# --------------------------------------------------------------------------


def _build(compute=COMPUTE, h=H, r=R, reps=1, hbufs=HBUFS, ybufs=2):
    """Build the Bass program for one core: x [P,h+2,W+2] -> o [P,h,W].

    v3: 5 DVE instructions per block, all 2 elems/cycle:
      W_p = HDIL2X(xt rows p..p+r-1)            p = 0,1,2
      M   = PMAX2C(W_0 + c01, W_1 + c11)
      o   = PMAX2C(M + 0,     W_2 + c21)
    ACT/PE/GPSIMD idle; DVE ~2.5 cyc/elem.
    """
    import concourse.bacc as bacc
    import concourse.mybir as mybir
    from concourse.tile import TileContext

    hdil2x, pmax2c = _get_v3_ops()
    dt = mybir.dt.float16 if compute == "f16" else mybir.dt.float32
    f32 = mybir.dt.float32

    nc = bacc.Bacc()
    x_d = nc.dram_tensor("x", [P, h + 2, W + 2], dt, kind="ExternalInput")
    # se2[:, 3p+0] = se[c,p,1]; 3p+1 = se[c,p,2]-se[c,p,1]; 3p+2 = se[c,p,0]-se[c,p,1]
    se_d = nc.dram_tensor("se", [P, K * K], f32, kind="ExternalInput")
    o_d = nc.dram_tensor("o", [P, h, W], dt, kind="ExternalOutput")

    WP_ = W + 2

    def pm(inst):
        getattr(inst, "ins", inst).perf_max = 1
        return inst

    with TileContext(nc) as tc:
        with (
            tc.tile_pool(name="cpool", bufs=1) as cpool,
            tc.tile_pool(name="xpool", bufs=2) as xpool,
            tc.tile_pool(name="wpool", bufs=hbufs) as wpool,
            tc.tile_pool(name="opool", bufs=2) as opool,
        ):
            se_sb = cpool.tile([P, K * K], f32)
            nc.sync.dma_start(out=se_sb[:], in_=se_d[:, :])

            L = r * WP_
            for r0 in [v for _ in range(reps) for v in range(0, h, r)]:
                xt = xpool.tile([P, r + 2, WP_], dt, tag="xt")
                nc.sync.dma_start(out=xt[:], in_=x_d[:, r0 : r0 + r + 2, :])
                xf = xt[:].rearrange("p h w -> p (h w)")

                def flat(tile):
                    return tile[:].rearrange("p h w -> p (h w)")

                ws = []
                for p in range(3):
                    wt = wpool.tile([P, r, WP_], dt, tag=f"w{p}")
                    off = p * WP_
                    pm(nc.vector._custom_dve(
                        hdil2x,
                        out=flat(wt)[:, 2:L],
                        in0=xf[:, off + 2 : off + L],
                        in1=xf[:, off : off + L - 2],
                        s0=se_sb[:, 3 * p + 1 : 3 * p + 2],
                        s1=se_sb[:, 3 * p + 2 : 3 * p + 3]))
                    ws.append(wt)

                mt = wpool.tile([P, r, WP_], dt, tag="m")
                pm(nc.vector._custom_dve(
                    pmax2c, out=flat(mt), in0=flat(ws[0]), in1=flat(ws[1]),
                    s0=se_sb[:, 0:1], s1=se_sb[:, 3:4]))
                ot = opool.tile([P, r, WP_], dt, tag="o")
                pm(nc.vector._custom_dve(
                    pmax2c, out=flat(ot), in0=flat(mt), in1=flat(ws[2]),
                    s0=0.0, s1=se_sb[:, 6:7]))

                nc.sync.dma_start(out=o_d[:, r0 : r0 + r, :],
                                  in_=ot[:, :, 2 : W + 2])
    nc.finalize()
    return nc


def _get_prog(key=("default",)):
    if key not in _prog_cache:
        _prog_cache[key] = _build()
    return _prog_cache[key]


def _pad_shard(x_shard, np_dt):
    """[BPC,C,H,W] fp32 -> zero-padded [P, H+2, W+2] in np_dt."""
    xp = np.zeros((P, HP, WP), np_dt)
    xp[:, 1 : H + 1, 1 : W + 1] = x_shard.reshape(P, H, W)
    return xp


def _prep_se(se):
    """se [C,3,3] fp32 -> derived per-partition consts [P, 9] fp32."""
    se = np.asarray(se, np.float32)
    d = np.empty((C, K * K), np.float32)
    for p in range(K):
        d[:, 3 * p + 0] = se[:, p, 1]
        d[:, 3 * p + 1] = se[:, p, 2] - se[:, p, 1]
        d[:, 3 * p + 2] = se[:, p, 0] - se[:, p, 1]
    return np.tile(d, (BPC, 1))


def _run(x, se, **spmd_kwargs):
    from concourse.bass_utils import run_bass_kernel_spmd

    nc = _get_prog()
    np_dt = np.float16 if COMPUTE == "f16" else np.float32
    x = np.asarray(x)
    se_p = _prep_se(se)
    in_maps = [
        {"x": _pad_shard(x[k * BPC : (k + 1) * BPC], np_dt), "se": se_p}
        for k in range(NCORES)
    ]
    res = run_bass_kernel_spmd(nc, in_maps, core_ids=list(range(NCORES)), **spmd_kwargs)
    out = np.empty((B, C, H, W), np.float32)
    for k in range(NCORES):
        out[k * BPC : (k + 1) * BPC] = (
            res.results[k]["o"].astype(np.float32).reshape(BPC, C, H, W)
        )
    return out, res


def kernel(x: np.ndarray, se: np.ndarray) -> np.ndarray:
    return _run(x, se)[0]


# revision 12
# speedup vs baseline: 1.9065x; 1.0015x over previous
"""Depthwise morphological (max-plus) dilation, 3x3, stride 1, zero-pad 1.

out[b,c,i,j] = max_{p,q} ( x_pad[b,c,i+p,j+q] + se[c,p,q] )

Sharding: pure data parallel over batch (16 batches -> 8 cores x 2).
On-core layout: partition dim = 2 batches x 64 channels = 128 planes;
each partition processes its own plane in row-blocks of R output rows.
The host supplies x zero-padded to [P, H+2, W+2] in fp16 plus a derived
per-partition SE tensor (middle-tap biases + outer-tap deltas), so the
device does no zero-fill.

All compute runs as 5 hand-written custom-DVE instructions per block,
each at the DVE's 2X_1PORT rate (2 elems/cycle):
  W_p = HDIL2X(x rows p..p+r-1)   = max(x[k]+d_p2, x[k-1], x[k-2]+d_p0)
        for p = 0,1,2  (src1 = the same stream shifted by -2 elements,
        so the window taps come from SRC_1/SRC_1_HI with no cross-cycle
        state; 8 ALU blocks per 2 elements)
  M    = PMAX2C(W_0 + c01, W_1 + c11)       (pointwise max-plus-const)
  out  = PMAX2C(M + 0,     W_2 + c21)
The uop programs are registered into concourse.dve_ops at build time and
packed into the NEFF's per-kernel DVE table; emitted instructions set
perf_max=1 so the engine engages the 2x slot (fp16 / stride-1 /
4B-aligned streams are guaranteed by construction, with an exact-ish 1x
fallback program in slot 0).

DVE: 2.5 cyc/elem (~172us/core, read-port and 8-ALU-block bound); ACT,
PE, GPSIMD idle; DMA ~95us/core fully hidden. Compare ~325us/core for
the best stock-op split (9 adds + 8 tensor-tensor maxes across
DVE 4x/2x perf modes + ACT) and ~205us for 1x-mode fused windowed ops.

fp16 compute gives ~9e-4 rel err vs the fp32 reference (scale ~6).
"""

import numpy as np

B, C, H, W = 16, 64, 256, 256
K = 3
NCORES = 8
BPC = B // NCORES          # batches per core
P = BPC * C                # 128 partitions
HP, WP = H + 2, W + 2      # host-padded plane

COMPUTE = "f16"            # "f16" (fast, ~2e-3 abs err) or "f32" (exact)
R = 32                     # output rows per block
HBUFS = 2                  # bufs for the W/M intermediate tiles

_prog_cache = {}

class _HandDveOp:
    """Duck-typed stand-in for dve_ops.DveOp with hand-written uops."""

    def __init__(self, name, spec, built, subdim=False):
        self.name, self.spec, self.subdim = name, spec, subdim
        self._built = built

    def compile(self, ver):
        assert ver == "v3", f"hand op {self.name} only built for v3, got {ver}"
        return self._built


# --------------------------------------------------------------------------
# v3: 2x-perf-mode ops.
#   HDIL2X_ANT: out[k] = max(a[k]+C0, y[k-1], y[k-2]+C1) with a = y[2:],
#     b = y[:-2] passed as the two streams (shift-view, no cross-cycle
#     state) -> fits a 2-elems/cycle uop program (8 ALU blocks / cycle).
#   PMAX2C_ANT: out[k] = max(a[k]+C0, b[k]+C1), 2 elems/cycle.
# Emitted instructions get .ins.perf_max = 1 so the engine engages the
# 2X_1PORT slot (fp16, stride-1, 4B-aligned streams guaranteed below).
# --------------------------------------------------------------------------


def _mk_v3_uops():
    from concourse.dve_uop import (
        UopConfig, AluOp, AluInp, InpSel, OutSel, OutPath, Trigger,
        DelayInp, ENABLE,
    )
    D = AluInp.PREV_DELAY_0

    def base(lanes):
        u = UopConfig()
        for src, lane in lanes:
            u.enable_input(src, lane)
        u.require_inp0 = ENABLE
        u.require_inp1 = ENABLE
        u.trigger = (Trigger.SRC_TENSOR_DONE, Trigger.NONE, Trigger.NONE)
        return u

    two_src = [(InpSel.SRC_0, 1), (InpSel.SRC_0_HI, 2), (InpSel.SRC_1, 3),
               (InpSel.SRC_1_HI, 4), (InpSel.CONST_0, 5), (InpSel.CONST_1, 6)]
    one_src = [(InpSel.SRC_0, 1), (InpSel.SRC_1, 3),
               (InpSel.CONST_0, 5), (InpSel.CONST_1, 6)]

    # HDIL2X 2x: c0=y[k] c1=y[k+1] c2=y[k-2] c3=y[k-1] c4=C0 c5=C1
    u = base(two_src)
    dp = u.datapath_config
    dp[0].enable_alu(AluOp.ADD, AluInp(D + 3), AluInp(D + 5))
    dp[0].pass_through_delay(0, 1, 2, 3, 4, 5)
    dp[1].enable_alu(AluOp.ADD, AluInp(D + 2), AluInp(D + 5))
    dp[1].pass_through_delay(0, 1, 3, 4)
    dp[1].enable_delay_from_src(DelayInp.PREV_ALU_OUT, 2)   # c2 <- a4
    dp[2].enable_alu(AluOp.MAX, AluInp.PREV_ALU_OUT, AluInp(D + 3))
    dp[2].pass_through_delay(0, 1, 2, 4)
    dp[3].enable_alu(AluOp.ADD, AluInp(D + 0), AluInp(D + 4))
    dp[3].pass_through_delay(0, 1, 2, 4)
    dp[3].enable_delay_from_src(DelayInp.PREV_ALU_OUT, 3)   # c3 <- m1
    dp[4].enable_alu(AluOp.MAX, AluInp.PREV_ALU_OUT, AluInp(D + 3))
    dp[4].pass_through_delay(0, 1, 2, 4)
    dp[5].enable_alu(AluOp.ADD, AluInp(D + 1), AluInp(D + 4))
    dp[5].pass_through_delay(0, 2)
    dp[5].enable_delay_from_src(DelayInp.PREV_ALU_OUT, 1)   # c1 <- OUT_k
    dp[6].enable_alu(AluOp.MAX, AluInp(D + 2), AluInp(D + 0))
    dp[6].pass_through_delay(1)
    dp[6].enable_delay_from_src(DelayInp.PREV_ALU_OUT, 2)   # c2 <- a3
    dp[7].enable_alu(AluOp.MAX, AluInp.PREV_ALU_OUT, AluInp(D + 2))
    dp[7].pass_through_delay(1)
    u.enable_output(OutSel.DELAY_1, OutPath.WR0_LO)
    u.enable_output(OutSel.ALU_OUT, OutPath.WR0_HI)
    hdil_2x = u

    # HDIL2X 1x fallback (first output stale)
    u = base(one_src)
    dp = u.datapath_config
    dp[0].op = AluOp.BYPASS
    dp[0].alu_src0 = AluInp.CURR_SWAP_OUT
    dp[0].alu_src1 = AluInp(D + 0)
    dp[0].alu_out_enable = ENABLE
    dp[0].swap_enable = ENABLE
    dp[0].pass_through_delay(0, 2, 4, 5)
    dp[1].enable_alu(AluOp.ADD, AluInp(D + 2), AluInp(D + 5))
    dp[1].pass_through_delay(0, 4)
    dp[1].enable_delay_from_src(DelayInp.PREV_ALU_OUT, 2)
    dp[2].enable_alu(AluOp.MAX, AluInp.PREV_ALU_OUT, AluInp(D + 2))
    dp[2].pass_through_delay(0, 4)
    dp[3].enable_alu(AluOp.ADD, AluInp(D + 0), AluInp(D + 4))
    dp[3].enable_delay_from_src(DelayInp.PREV_ALU_OUT, 2)
    dp[4].enable_alu(AluOp.MAX, AluInp.PREV_ALU_OUT, AluInp(D + 2))
    for st in (5, 6, 7):
        dp[st].pass_through_alu()
    u.enable_output(OutSel.ALU_OUT, OutPath.WR0_LO)
    hdil_1x = u

    # PMAX2C 2x
    u = base(two_src)
    dp = u.datapath_config
    dp[0].enable_alu(AluOp.ADD, AluInp(D + 0), AluInp(D + 4))
    dp[0].pass_through_delay(1, 2, 3, 4, 5)
    dp[1].enable_alu(AluOp.ADD, AluInp(D + 2), AluInp(D + 5))
    dp[1].pass_through_delay(1, 3, 4, 5)
    dp[1].enable_delay_from_src(DelayInp.PREV_ALU_OUT, 0)   # c0 <- p0
    dp[2].enable_alu(AluOp.MAX, AluInp.PREV_ALU_OUT, AluInp(D + 0))
    dp[2].pass_through_delay(1, 3, 4, 5)
    dp[3].enable_alu(AluOp.ADD, AluInp(D + 1), AluInp(D + 4))
    dp[3].pass_through_delay(3, 5)
    dp[3].enable_delay_from_src(DelayInp.PREV_ALU_OUT, 0)   # c0 <- OUT_k
    dp[4].enable_alu(AluOp.ADD, AluInp(D + 3), AluInp(D + 5))
    dp[4].pass_through_delay(0)
    dp[4].enable_delay_from_src(DelayInp.PREV_ALU_OUT, 1)   # c1 <- p1
    dp[5].enable_alu(AluOp.MAX, AluInp.PREV_ALU_OUT, AluInp(D + 1))
    dp[5].pass_through_delay(0)
    for st in (6, 7):
        dp[st].pass_through_alu()
        dp[st].pass_through_delay(0)
    u.enable_output(OutSel.DELAY_0, OutPath.WR0_LO)
    u.enable_output(OutSel.ALU_OUT, OutPath.WR0_HI)
    pmax_2x = u

    # PMAX2C 1x fallback (exact)
    u = base(one_src)
    dp = u.datapath_config
    dp[0].enable_alu(AluOp.ADD, AluInp(D + 0), AluInp(D + 4))
    dp[0].pass_through_delay(2, 5)
    dp[1].enable_alu(AluOp.ADD, AluInp(D + 2), AluInp(D + 5))
    dp[1].enable_delay_from_src(DelayInp.PREV_ALU_OUT, 0)
    dp[2].enable_alu(AluOp.MAX, AluInp.PREV_ALU_OUT, AluInp(D + 0))
    for st in (3, 4, 5, 6, 7):
        dp[st].pass_through_alu()
    u.enable_output(OutSel.ALU_OUT, OutPath.WR0_LO)
    pmax_1x = u

    return hdil_1x, hdil_2x, pmax_1x, pmax_2x


def _hdil2x_ref(in0, in1, s0, s1, imm2):
    c0 = np.asarray(s0)[..., None]
    c1 = np.asarray(s1)[..., None]
    mid = np.empty_like(in0)
    mid[..., :-1] = in1[..., 1:]
    mid[..., -1] = in0[..., -2]
    return np.maximum(np.maximum(in0 + c0, mid), in1 + c1)


def _pmax2c_ref(in0, in1, s0, s1, imm2):
    return np.maximum(in0 + np.asarray(s0)[..., None],
                      in1 + np.asarray(s1)[..., None])


def _get_v3_ops():
    import concourse.dve_ops as dve_ops_mod
    from concourse.dve_spec import Spec, Src0, Src1
    from concourse.dve_uop import DveOpSpec

    hdil_1x, hdil_2x, pmax_1x, pmax_2x = _mk_v3_uops()
    out = []
    for name, u1, u2, ref in (("HDIL2X_ANT", hdil_1x, hdil_2x, _hdil2x_ref),
                              ("PMAX2C_ANT", pmax_1x, pmax_2x, _pmax2c_ref)):
        if name in dve_ops_mod._SUB_OPCODE_FOR_NAME:
            out.append(next(o for o in dve_ops_mod.OPS if o.name == name))
            continue
        row = max(dve_ops_mod._SUB_OPCODE_FOR_NAME.values()) + 1
        assert row < 0x20
        built = DveOpSpec(name=name, uops=[u1], uops_2x=[u2],
                          opcode=row, rd1_en=True, perf_max=1)
        built.validate("v3")
        op = _HandDveOp(name, Spec(body=Src0 + Src1, reference=ref), built)
        dve_ops_mod.OPS.append(op)
        dve_ops_mod._SUB_OPCODE_FOR_NAME[name] = row
        dve_ops_mod.CUSTOM_DVE_SPECS[name] = op.spec
        out.append(op)
    return out


# --------------------------------------------------------------------------
# Bass program
# --------------------------------------------------------------------------


def _build(compute=COMPUTE, h=H, r=R, reps=1, hbufs=HBUFS, ybufs=2):
    """Build the Bass program for one core: x [P,h+2,W+2] -> o [P,h,W].

    v3: 5 DVE instructions per block, all 2 elems/cycle:
      W_p = HDIL2X(xt rows p..p+r-1)            p = 0,1,2
      M   = PMAX2C(W_0 + c01, W_1 + c11)
      o   = PMAX2C(M + 0,     W_2 + c21)
    ACT/PE/GPSIMD idle; DVE ~2.5 cyc/elem.
    """
    import concourse.bacc as bacc
    import concourse.mybir as mybir
    from concourse.tile import TileContext

    hdil2x, pmax2c = _get_v3_ops()
    dt = mybir.dt.float16 if compute == "f16" else mybir.dt.float32
    f32 = mybir.dt.float32

    nc = bacc.Bacc()
    x_d = nc.dram_tensor("x", [P, h + 2, W + 2], dt, kind="ExternalInput")
    # se2[:, 3p+0] = se[c,p,1]; 3p+1 = se[c,p,2]-se[c,p,1]; 3p+2 = se[c,p,0]-se[c,p,1]
    se_d = nc.dram_tensor("se", [P, K * K], f32, kind="ExternalInput")
    o_d = nc.dram_tensor("o", [P, h, W], dt, kind="ExternalOutput")

    WP_ = W + 2

    def pm(inst):
        getattr(inst, "ins", inst).perf_max = 1
        return inst

    with TileContext(nc) as tc:
        with (
            tc.tile_pool(name="cpool", bufs=1) as cpool,
            tc.tile_pool(name="xpool", bufs=2) as xpool,
            tc.tile_pool(name="wpool", bufs=hbufs) as wpool,
            tc.tile_pool(name="opool", bufs=2) as opool,
        ):
            se_sb = cpool.tile([P, K * K], f32)
            nc.sync.dma_start(out=se_sb[:], in_=se_d[:, :])

            L = r * WP_
            for r0 in [v for _ in range(reps) for v in range(0, h, r)]:
                xt = xpool.tile([P, r + 2, WP_], dt, tag="xt")
                nc.sync.dma_start(out=xt[:], in_=x_d[:, r0 : r0 + r + 2, :])
                xf = xt[:].rearrange("p h w -> p (h w)")

                def flat(tile):
                    return tile[:].rearrange("p h w -> p (h w)")

                ws = []
                for p in range(3):
                    wt = wpool.tile([P, r, WP_], dt, tag=f"w{p}")
                    off = p * WP_
                    pm(nc.vector._custom_dve(
                        hdil2x,
                        out=flat(wt)[:, 2:L],
                        in0=xf[:, off + 2 : off + L],
                        in1=xf[:, off : off + L - 2],
                        s0=se_sb[:, 3 * p + 1 : 3 * p + 2],
                        s1=se_sb[:, 3 * p + 2 : 3 * p + 3]))
                    ws.append(wt)

                mt = wpool.tile([P, r, WP_], dt, tag="m")
                pm(nc.vector._custom_dve(
                    pmax2c, out=flat(mt), in0=flat(ws[0]), in1=flat(ws[1]),
                    s0=se_sb[:, 0:1], s1=se_sb[:, 3:4]))
                ot = opool.tile([P, r, WP_], dt, tag="o")
                pm(nc.vector._custom_dve(
                    pmax2c, out=flat(ot), in0=flat(mt), in1=flat(ws[2]),
                    s0=0.0, s1=se_sb[:, 6:7]))

                nc.sync.dma_start(out=o_d[:, r0 : r0 + r, :],
                                  in_=ot[:, :, 2 : W + 2])
    nc.finalize()
    return nc


def _get_prog(key=("default",)):
    if key not in _prog_cache:
        _prog_cache[key] = _build()
    return _prog_cache[key]


def _pad_shard(x_shard, np_dt):
    """[BPC,C,H,W] fp32 -> zero-padded [P, H+2, W+2] in np_dt."""
    xp = np.zeros((P, HP, WP), np_dt)
    xp[:, 1 : H + 1, 1 : W + 1] = x_shard.reshape(P, H, W)
    return xp


def _prep_se(se):
    """se [C,3,3] fp32 -> derived per-partition consts [P, 9] fp32."""
    se = np.asarray(se, np.float32)
    d = np.empty((C, K * K), np.float32)
    for p in range(K):
        d[:, 3 * p + 0] = se[:, p, 1]
        d[:, 3 * p + 1] = se[:, p, 2] - se[:, p, 1]
        d[:, 3 * p + 2] = se[:, p, 0] - se[:, p, 1]
    return np.tile(d, (BPC, 1))


def _run(x, se, **spmd_kwargs):
    from concourse.bass_utils import run_bass_kernel_spmd

    nc = _get_prog()
    np_dt = np.float16 if COMPUTE == "f16" else np.float32
    x = np.asarray(x)
    se_p = _prep_se(se)
    in_maps = [
        {"x": _pad_shard(x[k * BPC : (k + 1) * BPC], np_dt), "se": se_p}
        for k in range(NCORES)
    ]
    res = run_bass_kernel_spmd(nc, in_maps, core_ids=list(range(NCORES)), **spmd_kwargs)
    out = np.empty((B, C, H, W), np.float32)
    for k in range(NCORES):
        out[k * BPC : (k + 1) * BPC] = (
            res.results[k]["o"].astype(np.float32).reshape(BPC, C, H, W)
        )
    return out, res


def kernel(x: np.ndarray, se: np.ndarray) -> np.ndarray:
    return _run(x, se)[0]


# revision 17
# speedup vs baseline: 1.9125x; 1.0031x over previous
"""Depthwise morphological (max-plus) dilation, 3x3, stride 1, zero-pad 1.

out[b,c,i,j] = max_{p,q} ( x_pad[b,c,i+p,j+q] + se[c,p,q] )

Sharding: pure data parallel over batch (16 batches -> 8 cores x 2).
On-core layout: partition dim = 2 batches x 64 channels = 128 planes;
each partition processes its own plane in row-blocks of R output rows.
The host supplies x zero-padded to [P, H+2, W+2] in fp16 plus a derived
per-partition SE tensor (middle-tap biases + outer-tap deltas), so the
device does no zero-fill.

All compute runs as 5 hand-written custom-DVE instructions per block,
each at the DVE's 2X_1PORT rate (2 elems/cycle):
  W_p = HDIL2X(x rows p..p+r-1)   = max(x[k]+d_p2, x[k-1], x[k-2]+d_p0)
        for p = 0,1,2  (src1 = the same stream shifted by -2 elements,
        so the window taps come from SRC_1/SRC_1_HI with no cross-cycle
        state; 8 ALU blocks per 2 elements)
  M    = PMAX2C(W_0 + c01, W_1 + c11)       (pointwise max-plus-const)
  out  = PMAX2C(M + 0,     W_2 + c21)
The uop programs are registered into concourse.dve_ops at build time and
packed into the NEFF's per-kernel DVE table; emitted instructions set
perf_max=1 so the engine engages the 2x slot (fp16 / stride-1 /
4B-aligned streams are guaranteed by construction, with an exact-ish 1x
fallback program in slot 0).

DVE: 2.5 cyc/elem (~172us/core, read-port and 8-ALU-block bound); ACT,
PE, GPSIMD idle; DMA ~95us/core fully hidden. Compare ~325us/core for
the best stock-op split (9 adds + 8 tensor-tensor maxes across
DVE 4x/2x perf modes + ACT) and ~205us for 1x-mode fused windowed ops.

fp16 compute gives ~9e-4 rel err vs the fp32 reference (scale ~6).
"""

import numpy as np

B, C, H, W = 16, 64, 256, 256
K = 3
NCORES = 8
BPC = B // NCORES          # batches per core
P = BPC * C                # 128 partitions
HP, WP = H + 2, W + 2      # host-padded plane

COMPUTE = "f16"            # "f16" (fast, ~2e-3 abs err) or "f32" (exact)
R = 32                     # output rows per block
HBUFS = 2                  # bufs for the W/M intermediate tiles

_prog_cache = {}

class _HandDveOp:
    """Duck-typed stand-in for dve_ops.DveOp with hand-written uops."""

    def __init__(self, name, spec, built, subdim=False):
        self.name, self.spec, self.subdim = name, spec, subdim
        self._built = built

    def compile(self, ver):
        assert ver == "v3", f"hand op {self.name} only built for v3, got {ver}"
        return self._built


# --------------------------------------------------------------------------
# v3: 2x-perf-mode ops.
#   HDIL2X_ANT: out[k] = max(a[k]+C0, y[k-1], y[k-2]+C1) with a = y[2:],
#     b = y[:-2] passed as the two streams (shift-view, no cross-cycle
#     state) -> fits a 2-elems/cycle uop program (8 ALU blocks / cycle).
#   PMAX2C_ANT: out[k] = max(a[k]+C0, b[k]+C1), 2 elems/cycle.
# Emitted instructions get .ins.perf_max = 1 so the engine engages the
# 2X_1PORT slot (fp16, stride-1, 4B-aligned streams guaranteed below).
# --------------------------------------------------------------------------


def _mk_v3_uops():
    from concourse.dve_uop import (
        UopConfig, AluOp, AluInp, InpSel, OutSel, OutPath, Trigger,
        DelayInp, ENABLE,
    )
    D = AluInp.PREV_DELAY_0

    def base(lanes):
        u = UopConfig()
        for src, lane in lanes:
            u.enable_input(src, lane)
        u.require_inp0 = ENABLE
        u.require_inp1 = ENABLE
        u.trigger = (Trigger.SRC_TENSOR_DONE, Trigger.NONE, Trigger.NONE)
        return u

    two_src = [(InpSel.SRC_0, 1), (InpSel.SRC_0_HI, 2), (InpSel.SRC_1, 3),
               (InpSel.SRC_1_HI, 4), (InpSel.CONST_0, 5), (InpSel.CONST_1, 6)]
    one_src = [(InpSel.SRC_0, 1), (InpSel.SRC_1, 3),
               (InpSel.CONST_0, 5), (InpSel.CONST_1, 6)]

    # HDIL2X 2x: c0=y[k] c1=y[k+1] c2=y[k-2] c3=y[k-1] c4=C0 c5=C1
    u = base(two_src)
    dp = u.datapath_config
    dp[0].enable_alu(AluOp.ADD, AluInp(D + 3), AluInp(D + 5))
    dp[0].pass_through_delay(0, 1, 2, 3, 4, 5)
    dp[1].enable_alu(AluOp.ADD, AluInp(D + 2), AluInp(D + 5))
    dp[1].pass_through_delay(0, 1, 3, 4)
    dp[1].enable_delay_from_src(DelayInp.PREV_ALU_OUT, 2)   # c2 <- a4
    dp[2].enable_alu(AluOp.MAX, AluInp.PREV_ALU_OUT, AluInp(D + 3))
    dp[2].pass_through_delay(0, 1, 2, 4)
    dp[3].enable_alu(AluOp.ADD, AluInp(D + 0), AluInp(D + 4))
    dp[3].pass_through_delay(0, 1, 2, 4)
    dp[3].enable_delay_from_src(DelayInp.PREV_ALU_OUT, 3)   # c3 <- m1
    dp[4].enable_alu(AluOp.MAX, AluInp.PREV_ALU_OUT, AluInp(D + 3))
    dp[4].pass_through_delay(0, 1, 2, 4)
    dp[5].enable_alu(AluOp.ADD, AluInp(D + 1), AluInp(D + 4))
    dp[5].pass_through_delay(0, 2)
    dp[5].enable_delay_from_src(DelayInp.PREV_ALU_OUT, 1)   # c1 <- OUT_k
    dp[6].enable_alu(AluOp.MAX, AluInp(D + 2), AluInp(D + 0))
    dp[6].pass_through_delay(1)
    dp[6].enable_delay_from_src(DelayInp.PREV_ALU_OUT, 2)   # c2 <- a3
    dp[7].enable_alu(AluOp.MAX, AluInp.PREV_ALU_OUT, AluInp(D + 2))
    dp[7].pass_through_delay(1)
    u.enable_output(OutSel.DELAY_1, OutPath.WR0_LO)
    u.enable_output(OutSel.ALU_OUT, OutPath.WR0_HI)
    hdil_2x = u

    # HDIL2X 1x fallback (first output stale)
    u = base(one_src)
    dp = u.datapath_config
    dp[0].op = AluOp.BYPASS
    dp[0].alu_src0 = AluInp.CURR_SWAP_OUT
    dp[0].alu_src1 = AluInp(D + 0)
    dp[0].alu_out_enable = ENABLE
    dp[0].swap_enable = ENABLE
    dp[0].pass_through_delay(0, 2, 4, 5)
    dp[1].enable_alu(AluOp.ADD, AluInp(D + 2), AluInp(D + 5))
    dp[1].pass_through_delay(0, 4)
    dp[1].enable_delay_from_src(DelayInp.PREV_ALU_OUT, 2)
    dp[2].enable_alu(AluOp.MAX, AluInp.PREV_ALU_OUT, AluInp(D + 2))
    dp[2].pass_through_delay(0, 4)
    dp[3].enable_alu(AluOp.ADD, AluInp(D + 0), AluInp(D + 4))
    dp[3].enable_delay_from_src(DelayInp.PREV_ALU_OUT, 2)
    dp[4].enable_alu(AluOp.MAX, AluInp.PREV_ALU_OUT, AluInp(D + 2))
    for st in (5, 6, 7):
        dp[st].pass_through_alu()
    u.enable_output(OutSel.ALU_OUT, OutPath.WR0_LO)
    hdil_1x = u

    # PMAX2C 2x
    u = base(two_src)
    dp = u.datapath_config
    dp[0].enable_alu(AluOp.ADD, AluInp(D + 0), AluInp(D + 4))
    dp[0].pass_through_delay(1, 2, 3, 4, 5)
    dp[1].enable_alu(AluOp.ADD, AluInp(D + 2), AluInp(D + 5))
    dp[1].pass_through_delay(1, 3, 4, 5)
    dp[1].enable_delay_from_src(DelayInp.PREV_ALU_OUT, 0)   # c0 <- p0
    dp[2].enable_alu(AluOp.MAX, AluInp.PREV_ALU_OUT, AluInp(D + 0))
    dp[2].pass_through_delay(1, 3, 4, 5)
    dp[3].enable_alu(AluOp.ADD, AluInp(D + 1), AluInp(D + 4))
    dp[3].pass_through_delay(3, 5)
    dp[3].enable_delay_from_src(DelayInp.PREV_ALU_OUT, 0)   # c0 <- OUT_k
    dp[4].enable_alu(AluOp.ADD, AluInp(D + 3), AluInp(D + 5))
    dp[4].pass_through_delay(0)
    dp[4].enable_delay_from_src(DelayInp.PREV_ALU_OUT, 1)   # c1 <- p1
    dp[5].enable_alu(AluOp.MAX, AluInp.PREV_ALU_OUT, AluInp(D + 1))
    dp[5].pass_through_delay(0)
    for st in (6, 7):
        dp[st].pass_through_alu()
        dp[st].pass_through_delay(0)
    u.enable_output(OutSel.DELAY_0, OutPath.WR0_LO)
    u.enable_output(OutSel.ALU_OUT, OutPath.WR0_HI)
    pmax_2x = u

    # PMAX2C 1x fallback (exact)
    u = base(one_src)
    dp = u.datapath_config
    dp[0].enable_alu(AluOp.ADD, AluInp(D + 0), AluInp(D + 4))
    dp[0].pass_through_delay(2, 5)
    dp[1].enable_alu(AluOp.ADD, AluInp(D + 2), AluInp(D + 5))
    dp[1].enable_delay_from_src(DelayInp.PREV_ALU_OUT, 0)
    dp[2].enable_alu(AluOp.MAX, AluInp.PREV_ALU_OUT, AluInp(D + 0))
    for st in (3, 4, 5, 6, 7):
        dp[st].pass_through_alu()
    u.enable_output(OutSel.ALU_OUT, OutPath.WR0_LO)
    pmax_1x = u

    return hdil_1x, hdil_2x, pmax_1x, pmax_2x


def _hdil2x_ref(in0, in1, s0, s1, imm2):
    c0 = np.asarray(s0)[..., None]
    c1 = np.asarray(s1)[..., None]
    mid = np.empty_like(in0)
    mid[..., :-1] = in1[..., 1:]
    mid[..., -1] = in0[..., -2]
    return np.maximum(np.maximum(in0 + c0, mid), in1 + c1)


def _pmax2c_ref(in0, in1, s0, s1, imm2):
    return np.maximum(in0 + np.asarray(s0)[..., None],
                      in1 + np.asarray(s1)[..., None])


def _get_v3_ops():
    import concourse.dve_ops as dve_ops_mod
    from concourse.dve_spec import Spec, Src0, Src1
    from concourse.dve_uop import DveOpSpec

    hdil_1x, hdil_2x, pmax_1x, pmax_2x = _mk_v3_uops()
    out = []
    for name, u1, u2, ref in (("HDIL2X_ANT", hdil_1x, hdil_2x, _hdil2x_ref),
                              ("PMAX2C_ANT", pmax_1x, pmax_2x, _pmax2c_ref)):
        if name in dve_ops_mod._SUB_OPCODE_FOR_NAME:
            out.append(next(o for o in dve_ops_mod.OPS if o.name == name))
            continue
        row = max(dve_ops_mod._SUB_OPCODE_FOR_NAME.values()) + 1
        assert row < 0x20
        built = DveOpSpec(name=name, uops=[u1], uops_2x=[u2],
                          opcode=row, rd1_en=True, perf_max=1)
        built.validate("v3")
        op = _HandDveOp(name, Spec(body=Src0 + Src1, reference=ref), built)
        dve_ops_mod.OPS.append(op)
        dve_ops_mod._SUB_OPCODE_FOR_NAME[name] = row
        dve_ops_mod.CUSTOM_DVE_SPECS[name] = op.spec
        out.append(op)
    return out


# --------------------------------------------------------------------------
# Bass program
# --------------------------------------------------------------------------


def _build(compute=COMPUTE, h=H, r=R, reps=1, hbufs=HBUFS, ybufs=2):
    """Build the Bass program for one core: x [P,h+2,W+2] -> o [P,h,W].

    v3: 5 DVE instructions per block, all 2 elems/cycle:
      W_p = HDIL2X(xt rows p..p+r-1)            p = 0,1,2
      M   = PMAX2C(W_0 + c01, W_1 + c11)
      o   = PMAX2C(M + 0,     W_2 + c21)
    ACT/PE/GPSIMD idle; DVE ~2.5 cyc/elem.
    """
    import concourse.bacc as bacc
    import concourse.mybir as mybir
    from concourse.tile import TileContext

    hdil2x, pmax2c = _get_v3_ops()
    dt = mybir.dt.float16 if compute == "f16" else mybir.dt.float32
    f32 = mybir.dt.float32

    nc = bacc.Bacc()
    x_d = nc.dram_tensor("x", [P, h + 2, W + 2], dt, kind="ExternalInput")
    # se2[:, 3p+0] = se[c,p,1]; 3p+1 = se[c,p,2]-se[c,p,1]; 3p+2 = se[c,p,0]-se[c,p,1]
    se_d = nc.dram_tensor("se", [P, K * K], f32, kind="ExternalInput")
    o_d = nc.dram_tensor("o", [P, h, W], dt, kind="ExternalOutput")

    WP_ = W + 2

    def pm(inst):
        getattr(inst, "ins", inst).perf_max = 1
        return inst

    with TileContext(nc) as tc:
        with (
            tc.tile_pool(name="cpool", bufs=1) as cpool,
            tc.tile_pool(name="xpool", bufs=2) as xpool,
            tc.tile_pool(name="wpool", bufs=hbufs) as wpool,
            tc.tile_pool(name="opool", bufs=2) as opool,
        ):
            se_sb = cpool.tile([P, K * K], f32)
            nc.sync.dma_start(out=se_sb[:], in_=se_d[:, :])

            L = r * WP_
            for r0 in [v for _ in range(reps) for v in range(0, h, r)]:
                # xt has a 2-element header so every window instruction can
                # read src1 two elements behind src0 over the FULL output
                # range: all boundary/stale results land in the per-row junk
                # columns (tile cols 0,1), which are never stored. This keeps
                # the 1x fallback program exact too.
                xt = xpool.tile([P, (r + 2) * WP_ + 2], dt, tag="xt")
                nc.sync.dma_start(
                    out=xt[:, 2 : 2 + (r + 2) * WP_],
                    in_=x_d[:, r0 : r0 + r + 2, :].rearrange(
                        "p h w -> p (h w)"))
                xf = xt[:]

                def flat(tile):
                    return tile[:].rearrange("p h w -> p (h w)")

                ws = []
                for p in range(3):
                    wt = wpool.tile([P, r, WP_], dt, tag=f"w{p}")
                    off = p * WP_
                    pm(nc.vector._custom_dve(
                        hdil2x,
                        out=flat(wt),
                        in0=xf[:, off + 2 : off + 2 + L],
                        in1=xf[:, off : off + L],
                        s0=se_sb[:, 3 * p + 1 : 3 * p + 2],
                        s1=se_sb[:, 3 * p + 2 : 3 * p + 3]))
                    ws.append(wt)

                mt = wpool.tile([P, r, WP_], dt, tag="m")
                pm(nc.vector._custom_dve(
                    pmax2c, out=flat(mt), in0=flat(ws[0]), in1=flat(ws[1]),
                    s0=se_sb[:, 0:1], s1=se_sb[:, 3:4]))
                ot = opool.tile([P, r, WP_], dt, tag="o")
                pm(nc.vector._custom_dve(
                    pmax2c, out=flat(ot), in0=flat(mt), in1=flat(ws[2]),
                    s0=0.0, s1=se_sb[:, 6:7]))

                nc.sync.dma_start(out=o_d[:, r0 : r0 + r, :],
                                  in_=ot[:, :, 2 : W + 2])
    nc.finalize()
    return nc


def _get_prog(key=("default",)):
    if key not in _prog_cache:
        _prog_cache[key] = _build()
    return _prog_cache[key]


def _pad_shard(x_shard, np_dt):
    """[BPC,C,H,W] fp32 -> zero-padded [P, H+2, W+2] in np_dt."""
    xp = np.zeros((P, HP, WP), np_dt)
    xp[:, 1 : H + 1, 1 : W + 1] = x_shard.reshape(P, H, W)
    return xp


def _prep_se(se):
    """se [C,3,3] fp32 -> derived per-partition consts [P, 9] fp32."""
    se = np.asarray(se, np.float32)
    d = np.empty((C, K * K), np.float32)
    for p in range(K):
        d[:, 3 * p + 0] = se[:, p, 1]
        d[:, 3 * p + 1] = se[:, p, 2] - se[:, p, 1]
        d[:, 3 * p + 2] = se[:, p, 0] - se[:, p, 1]
    return np.tile(d, (BPC, 1))


def _run(x, se, **spmd_kwargs):
    from concourse.bass_utils import run_bass_kernel_spmd

    nc = _get_prog()
    np_dt = np.float16 if COMPUTE == "f16" else np.float32
    x = np.asarray(x)
    se_p = _prep_se(se)
    in_maps = [
        {"x": _pad_shard(x[k * BPC : (k + 1) * BPC], np_dt), "se": se_p}
        for k in range(NCORES)
    ]
    res = run_bass_kernel_spmd(nc, in_maps, core_ids=list(range(NCORES)), **spmd_kwargs)
    out = np.empty((B, C, H, W), np.float32)
    for k in range(NCORES):
        out[k * BPC : (k + 1) * BPC] = (
            res.results[k]["o"].astype(np.float32).reshape(BPC, C, H, W)
        )
    return out, res


def kernel(x: np.ndarray, se: np.ndarray) -> np.ndarray:
    return _run(x, se)[0]
